# revision 22
# baseline (speedup 1.0000x reference)
"""AttentionBlock (GroupNorm -> qkv -> softmax attention -> proj + residual)
for Trainium2, 8 NeuronCores, fp8 DoubleRow edition.

Sharding: core = (batch b, head-half hh): each core handles 1 of 4 batches
and 4 of 8 heads, computing a partial projection output; the host sums the
two partials per batch and adds the residual x and proj_b.

Device-side structure (per core):
 - GroupNorm is folded into the weights on the HOST: h = s*x + off with
   per-(batch,channel) s/off from exact f32 stats, so W' = W*diag(s) (fp8)
   and per-out-channel biases ride the PSUM->SBUF drains.  x ships as fp8.
 - ALL matmuls (qkv/scores/av/proj) run in fp8e4 DoubleRow perf mode
   (0.5 cycles/row).  The score matmuls have only a 64-deep contraction
   (per-head channels); DoubleRow needs a k-tile PAIR, so q/k live in
   [128, 2(kt), 4(mc), T] tiles whose kt=1 plane is zero: lhsT/rhs APs
   [64, 2, m] contract over (64 ch + 64 zeros) -- numerically exact, and
   the cost halves.
 - exp(scores) is split between the ACT engine (native Exp) and the DVE
   (a custom quartic c2*(c0*x+c1)^4 DVE op registered at import time).
 - softmax normalization: rowsums come free via a ones-column in v^T; per
   unit ONE fused reciprocal [1,2,512] (DVE), ONE partition_broadcast
   (GPSIMD) and ONE multiply [64,2,512] (DVE) cover both t-halves.

The mask input is all-True per the problem spec, a numeric no-op.  q/k/GN
biases are folded exactly; v/proj biases are added exactly on the host.
"""

import os
import numpy as np
import ml_dtypes

import concourse.bass as bass
import concourse.tile as tile
from concourse import bacc, mybir, library_config
from concourse.bass_utils import run_bass_kernel_spmd

F32 = mybir.dt.float32
BF16 = mybir.dt.bfloat16
FP8 = mybir.dt.float8e4
AF = mybir.ActivationFunctionType
ALU = mybir.AluOpType
DR = mybir.MatmulPerfMode.DoubleRow
E4 = ml_dtypes.float8_e4m3

B, C, T, H = 4, 512, 2048, 8
CH = 64
G = 32
EPS = 1e-5
HL = 4                 # heads per core
P = 128
TH = T // 2            # 1024, t-half handled per (hd, th)
N_CORES = 8

# fp8 range scales
SW = 64.0              # weight upscale for fp8 (qkv + proj weights)
SQ = 4.0               # q/k sbuf upscale (on top of 1/sqrt(sqrt(ch)))
SV = 64.0              # v sbuf upscale (=SW so vt drain is a plain copy)
GAMMA = 1.0 / (SQ * SQ)  # descale applied inside exp
SCALE = 1.0 / np.sqrt(np.sqrt(CH))

# quartic exp approximation constants (minimax on [-1.7, 1.7])
QC0, QC1, QC2 = 0.24274105, 1.02873227, 1.04374374

# exp engine split: chunk i of 16 per (hd, th) goes to ACT if pattern bit set
EXP_ACT_FRAC = float(os.environ.get("EXP_ACT_FRAC", "0.59"))
# drain engine choices ("act" or "dve")
QK_DRAIN = os.environ.get("QK_DRAIN", "alt")
VT_DRAIN = os.environ.get("VT_DRAIN", "act")
PJ_DRAIN = os.environ.get("PJ_DRAIN", "alt")
VT_POS = os.environ.get("VT_POS", "spread")
WPOOL = int(os.environ.get("WPOOL", "12"))
VT_SLOTS = tuple(int(v) for v in os.environ.get("VT_SLOTS", "1,3,5,7").split(","))
WARMUP = int(os.environ.get("WARMUP", "12"))
NORM_LAG = int(os.environ.get("NORM_LAG", "0"))
SPLIT_LAST = int(os.environ.get("SPLIT_LAST", "0"))
OUTP = int(os.environ.get("OUTP", "6"))
# PJ_POOL: where proj psum tiles come from. 0 = borrow the sps ring
# (stalls the score stream ~1us per burst); 2 = borrow the sh (avs) ring
# (proj naturally waits for the freshly-freed avs slot).  1 = dedicated
# pool, only with SPS_BUFS=2 — measured much worse, keep for reference.
PJ_POOL = int(os.environ.get("PJ_POOL", "2"))
SPS_BUFS = int(os.environ.get("SPS_BUFS", "2" if PJ_POOL == 1 else "3"))

# ---- custom DVE op: EXP4_ANT = c2*(c0*x+c1)^4 ------------------------------
from concourse import dve_ops as _dops
from concourse.dve_spec import Spec as _Spec, Src0 as _Src0, C0 as _C0, \
    C1 as _C1, C2 as _C2, sq as _sq, lower as _lower
from concourse.dve_uop import DveOpSpec as _DveOpSpec


def _exp4_ref(in0, in1, c0, c1, c2):
    y = np.square(np.square(in0.astype(np.float32) * c0 + c1)) * c2
    return y.astype(np.float32)


def _register_exp4():
    for op in _dops.OPS:
        if op.name == "EXP4_ANT":
            return op
    spec = _Spec(body=_sq(_sq(_Src0 * _C0 + _C1)) * _C2, reference=_exp4_ref)
    shas = {}
    for ver in ("v3", "v4"):
        s = _DveOpSpec(name="EXP4_ANT", opcode=0, uops=_lower(spec, ver=ver),
                       rd1_en=False)
        shas[ver] = s.sha(ver)
    op = _dops.DveOp("EXP4_ANT", spec, subdim=False, uops_sha=shas)
    _dops.OPS.append(op)
    _dops.CUSTOM_DVE_SPECS[op.name] = spec
    _dops._SUB_OPCODE_FOR_NAME[op.name] = (
        max(_dops._SUB_OPCODE_FOR_NAME.values()) + 1)
    return op


EXP4 = _register_exp4()


EXP_ACT_EARLY = float(os.environ.get("EXP_ACT_EARLY", "0.50"))
EXP_EARLY_CHUNKS = int(os.environ.get("EXP_EARLY_CHUNKS", "16"))
EXP_ACT_LATE = float(os.environ.get("EXP_ACT_LATE", "0.55"))
EXP_LATE_CHUNKS = int(os.environ.get("EXP_LATE_CHUNKS", "16"))


def _exp_engine_pattern():
    """One entry per exp chunk (128 total): True -> ACT, False -> DVE.
    Early chunks lean DVE (ACT busy with qkv drains); late chunks move
    toward 50/50 so both engines finish the last unit together."""
    total_act = EXP_ACT_FRAC * 128
    early_act = EXP_ACT_EARLY * EXP_EARLY_CHUNKS
    late_act = EXP_ACT_LATE * EXP_LATE_CHUNKS
    mid = 128 - EXP_EARLY_CHUNKS - EXP_LATE_CHUNKS
    mid_frac = (total_act - early_act - late_act) / mid
    pat = []
    acc = 0.0
    for i in range(128):
        if i < EXP_EARLY_CHUNKS:
            f = EXP_ACT_EARLY
        elif i >= 128 - EXP_LATE_CHUNKS:
            f = EXP_ACT_LATE
        else:
            f = mid_frac
        acc += f
        if acc >= 1.0 - 1e-9:
            acc -= 1.0
            pat.append(True)
        else:
            pat.append(False)
    return pat


def _build_nc():
    nc = bacc.Bacc(
        "TRN2",
        target_bir_lowering=False,
        debug=False,
        enable_asserts=False,
        num_devices=N_CORES,
    )
    x_d = nc.dram_tensor("x", [P, 4, T], FP8, kind="ExternalInput").ap()
    wqk_d = nc.dram_tensor("wqk", [P, 4, 4, P], FP8, kind="ExternalInput").ap()
    wv_d = nc.dram_tensor("wv", [P, 4, 2 * P], FP8, kind="ExternalInput").ap()
    wp_d = nc.dram_tensor("wp", [P, 2, 4, P], FP8, kind="ExternalInput").ap()
    cqk_d = nc.dram_tensor("cqk", [P, 4], F32, kind="ExternalInput").ap()
    out_d = nc.dram_tensor("out", [P, 4, T], BF16, kind="ExternalOutput").ap()

    pat = _exp_engine_pattern()

    with tile.TileContext(nc) as tc:
        with (
            tc.tile_pool(name="consts", bufs=1) as consts,
            tc.tile_pool(name="xp", bufs=1) as xp,
            tc.tile_pool(name="qkp", bufs=1) as qkp,
            tc.tile_pool(name="vtp", bufs=1) as vtp,
            tc.tile_pool(name="ap", bufs=1) as apool,
            tc.tile_pool(name="wpool", bufs=WPOOL) as wpool,
            tc.tile_pool(name="rhop", bufs=3) as rhop,
            tc.tile_pool(name="repp", bufs=3) as repp,
            tc.tile_pool(name="outp", bufs=OUTP) as outp,
            tc.tile_pool(name="ps_sps", bufs=SPS_BUFS, space="PSUM") as ps_sps,
            tc.tile_pool(name="ps_pj", bufs=1, space="PSUM") as ps_pj,
            tc.tile_pool(name="ps_sh", bufs=1, space="PSUM") as ps_sh,
        ):
            nc.gpsimd.load_library(library_config.attn)

            # ---- DMA in ----
            # all input DMAs issue from the GPSIMD queue: Pool DMA config
            # is 25ns/instr vs SP's 565, and the 16 DMA engines run the
            # transfers in parallel -- x (split in quarters) + wqk all land
            # by ~1.7us instead of ~4us.
            x_sb = xp.tile([P, 4, T], FP8)
            for xq in range(4):
                nc.gpsimd.dma_start(x_sb[:, :, xq * 512 : (xq + 1) * 512],
                                    x_d[:, :, xq * 512 : (xq + 1) * 512])
            wqk = consts.tile([P, 4, 4, P], FP8)
            nc.gpsimd.dma_start(wqk, wqk_d)
            cqk = consts.tile([P, 4], F32)
            nc.gpsimd.dma_start(cqk, cqk_d)
            wv = consts.tile([P, 4, 2 * P], FP8)
            nc.gpsimd.dma_start(wv, wv_d)
            wp = consts.tile([P, 2, 4, P], FP8)
            nc.gpsimd.dma_start(wp, wp_d)

            # PE p-state warmup while input DMAs land: dummy matmuls on a
            # const tile keep the PE continuously busy so real matmuls start
            # at full clock.
            warm = consts.tile([P, P], FP8)
            nc.vector.memset(warm, 0.0)
            warm2 = consts.tile([P, 512], FP8)
            nc.vector.memset(warm2, 0.0)
            warm_ps = ps_sps.tile([P, 512], F32, tag="sps", name="warm")
            for _ in range(WARMUP):
                nc.tensor.matmul(warm_ps[:, 0:128], lhsT=warm,
                                 rhs=warm2[:, 0:128], start=True, stop=True)

            # ---- qk matmuls + drains ----
            # qk_sb: [128, 2 (kt), 4 (mc), T] fp8.  kt=0 holds q/k data
            # (mc 0: q heads 0/1, 1: q heads 2/3, 2: k heads 0/1, 3: k
            # heads 2/3; head parity is the 64-partition band), kt=1 is
            # ZERO so score matmuls can run DoubleRow with APs
            # [64, 2(kt), m] -- contraction (64 ch + 64 zeros).
            qk_sb = qkp.tile([P, 2, 4, T], FP8)
            # zero the kt=1 planes on the (otherwise idle) GPSIMD engine,
            # in first-use order: k_a, q_a, k_b, q_b
            for mc in (2, 0, 3, 1):
                nc.gpsimd.memset(qk_sb[:, 1, mc, :], 0.0)

            def qk_group(mc, tc2):
                # fused [128, 1024] tile (two tc4 halves) in the sps pool
                qkt = ps_sps.tile([P, 2, 512], F32, tag="sps",
                                  name=f"qk{mc}{tc2}")
                for t2 in range(2):
                    tc4 = tc2 * 2 + t2
                    for kcp in range(2):
                        nc.tensor.matmul(
                            qkt[:, t2, :],
                            lhsT=wqk[:, 2 * kcp : 2 * kcp + 2, mc, :],
                            rhs=x_sb[:, 2 * kcp : 2 * kcp + 2,
                                     tc4 * 512 : (tc4 + 1) * 512],
                            start=(kcp == 0), stop=(kcp == 1),
                            perf_mode=DR,
                        )
                dst = qk_sb[:, 0, mc, tc2 * 1024 : (tc2 + 1) * 1024]
                if QK_DRAIN == "act" or (QK_DRAIN == "alt" and mc in (0, 1)) \
                        or (QK_DRAIN == "alt2" and mc in (2, 3)):
                    nc.scalar.activation(
                        dst,
                        qkt.rearrange("p a b -> p (a b)"),
                        AF.Identity,
                        bias=cqk[:, mc : mc + 1],
                        scale=float(SCALE * SQ / SW),
                    )
                else:
                    nc.vector.tensor_scalar(
                        dst,
                        qkt.rearrange("p a b -> p (a b)"),
                        float(SCALE * SQ / SW),
                        cqk[:, mc : mc + 1],
                        ALU.mult, ALU.add,
                    )

            # ---- vt matmuls + drains ----
            # vt_sb: [128 (s%128), 16 (sc), 4 (hd), 128] fp8; cols 64..127
            # are ONES so av rows 64..127 all come out as the rowsum -- a
            # 64-way replicated rowsum that feeds reciprocal directly (no
            # partition_broadcast needed).
            vt_sb = vtp.tile([P, 16, HL, 2 * CH], FP8)
            nc.gpsimd.memset(vt_sb[:, :, :, CH : 2 * CH], 1.0)

            def vt_group4(g):
                # fused tile: 4 sc chunks (= scp pair 2g, 2g+1)
                vtt = ps_sps.tile([P, 4, 2 * P], F32, tag="sps",
                                  name=f"vt{g}")
                for s4 in range(4):
                    sc = g * 4 + s4
                    for kcp in range(2):
                        nc.tensor.matmul(
                            vtt[:, s4, :],
                            lhsT=x_sb[:, 2 * kcp : 2 * kcp + 2,
                                      sc * P : (sc + 1) * P],
                            rhs=wv[:, 2 * kcp : 2 * kcp + 2, :],
                            start=(kcp == 0), stop=(kcp == 1),
                            perf_mode=DR,
                        )
                if VT_DRAIN == "act" or (VT_DRAIN == "alt" and g % 2 == 0):
                    nc.scalar.activation(
                        vt_sb[:, 4 * g : 4 * g + 4, :, 0:CH],
                        vtt.rearrange("p a (h c) -> p a h c", h=HL),
                        AF.Identity,
                    )
                else:
                    nc.vector.tensor_copy(
                        vt_sb[:, 4 * g : 4 * g + 4, :, 0:CH],
                        vtt.rearrange("p a (h c) -> p a h c", h=HL),
                    )

            # qk for heads 0,1 first so attention can start early
            for tc2 in range(2):
                qk_group(2, tc2)           # k_a
                qk_group(0, tc2)           # q_a
            for tc2 in range(2):
                qk_group(3, tc2)           # k_b
                qk_group(1, tc2)           # q_b
            if VT_POS == "pre":
                for g in range(4):
                    vt_group4(g)

            # ---- attention ----
            a_sb = apool.tile([P, 2, T], FP8)

            # attention as a software-pipelined chunk stream: av matmuls
            # lag the scores/exp stream by AV_LAG chunk-pairs so PE never
            # waits on the previous unit's last exp at unit boundaries.
            AV_LAG = int(os.environ.get("AV_LAG", "5"))
            units = [(hd, th) for th in range(2) for hd in range(HL)]
            state = {}   # u -> dict(avs, w_ts)
            exp_ctr = [0]

            def unit_geom(u):
                hd, th = units[u]
                b0 = 64 * (hd % 2)
                q_mc = 0 if hd < 2 else 1
                k_mc = 2 if hd < 2 else 3
                return hd, th, b0, q_mc, k_mc

            def emit_chunk(u, scp):
                hd, th, b0, q_mc, k_mc = unit_geom(u)
                toff = th * TH
                if scp == 0:
                    state[u] = dict(
                        avs=ps_sh.tile([P, 2, 512], F32, tag="sh",
                                       name=f"av{hd}{th}"),
                        w_ts={})
                w_t = wpool.tile([P, 2, TH], FP8, name="wt")
                state[u]["w_ts"][scp] = w_t
                split = u >= len(units) - SPLIT_LAST
                for j in range(2):
                    sc = scp * 2 + j
                    sps = ps_sps.tile([P, TH], F32, tag="sps", name="sps")
                    for tq in range(2):
                        nc.tensor.matmul(
                            sps[:, tq * 512 : (tq + 1) * 512],
                            lhsT=qk_sb[b0 : b0 + CH, :, k_mc,
                                       sc * P : (sc + 1) * P],
                            rhs=qk_sb[b0 : b0 + CH, :, q_mc,
                                      toff + tq * 512 : toff + (tq + 1) * 512],
                            start=True, stop=True,
                            perf_mode=DR,
                        )
                    if split:
                        # tail units: halve each chunk across BOTH engines so
                        # the slot frees sooner and av-tq halves unblock early
                        nc.scalar.activation(
                            w_t[:, j, 0:512], sps[:, 0:512], AF.Exp,
                            scale=float(GAMMA))
                        nc.vector._custom_dve(
                            EXP4, out=w_t[:, j, 512:TH], in0=sps[:, 512:TH],
                            s0=float(QC0 * GAMMA), s1=float(QC1),
                            imm2=float(QC2))
                    elif pat[exp_ctr[0]]:
                        nc.scalar.activation(
                            w_t[:, j, :], sps, AF.Exp, scale=float(GAMMA))
                    else:
                        nc.vector._custom_dve(
                            EXP4, out=w_t[:, j, :], in0=sps,
                            s0=float(QC0 * GAMMA), s1=float(QC1),
                            imm2=float(QC2))
                    exp_ctr[0] += 1

            def emit_av(u, scp):
                hd, th, b0, q_mc, k_mc = unit_geom(u)
                avs = state[u]["avs"]
                w_t = state[u]["w_ts"].pop(scp)
                for tq in range(2):
                    nc.tensor.matmul(
                        avs[:, tq, :],
                        lhsT=vt_sb[:, 2 * scp : 2 * scp + 2, hd, :],
                        rhs=w_t[:, :, tq * 512 : (tq + 1) * 512],
                        start=(scp == 0), stop=(scp == 7),
                        perf_mode=DR,
                    )

            def emit_normalize(u, between=None):
                hd, th, b0, q_mc, k_mc = unit_geom(u)
                toff = th * TH
                avs = state[u]["avs"]
                if between is None:
                    # fused across both tq halves: reciprocal of the 64
                    # replicated rowsum rows IS the broadcast recip; then
                    # one multiply [64,2,512]
                    rep = repp.tile([CH, 2, 512], F32, name="rep")
                    nc.vector.reciprocal(rep, avs[CH : 2 * CH, :, :])
                    nc.vector.tensor_tensor(
                        a_sb[CH * (hd % 2) : CH * (hd % 2) + CH, hd // 2,
                             toff : toff + TH],
                        avs[0:CH, :, :], rep, ALU.mult,
                    )
                else:
                    # last unit: both recips first (independent), then the
                    # tq0 multiply, proj_tc(2) interleaved, tq1 multiply
                    reps = []
                    for tq in range(2):
                        rep = repp.tile([CH, 512], F32, name="rep")
                        nc.vector.reciprocal(rep, avs[CH : 2 * CH, tq, :])
                        reps.append(rep)
                    for tq in range(2):
                        nc.vector.tensor_tensor(
                            a_sb[CH * (hd % 2) : CH * (hd % 2) + CH, hd // 2,
                                 toff + tq * 512 : toff + (tq + 1) * 512],
                            avs[0:CH, tq, :], reps[tq], ALU.mult,
                        )
                        if tq == 0:
                            between()
                del state[u]

            def run_attention(extra=()):
                stream = [(u, scp) for u in range(len(units))
                          for scp in range(8)]
                norm_q = []   # units whose avs are done, normalize deferred

                def pop_norm():
                    lu = norm_q.pop(0)
                    emit_normalize(lu)
                    if lu == 3:           # last th0 unit done
                        proj_tc(0)
                    elif lu == 5:
                        proj_tc(1)

                for g, (u, scp) in enumerate(stream):
                    emit_chunk(u, scp)
                    if VT_POS == "stream" and g < 4:
                        vt_group4(g)
                    elif VT_POS == "spread" and g in VT_SLOTS:
                        vt_group4(VT_SLOTS.index(g))
                    lag = g - AV_LAG
                    if lag >= 0:
                        lu, lscp = stream[lag]
                        emit_av(lu, lscp)
                        if lscp == 7:
                            norm_q.append(lu)
                    if norm_q:
                        lu = norm_q[0]
                        close_g = (lu * 8 + 7) + AV_LAG  # g when avs closed
                        if g >= close_g + NORM_LAG:
                            pop_norm()
                for lu, lscp in stream[-AV_LAG:]:
                    emit_av(lu, lscp)
                    if lscp == 7:
                        norm_q.append(lu)
                while len(norm_q) > 1:
                    pop_norm()
                emit_normalize(norm_q.pop(0),
                               between=lambda: proj_tc(2, tail=True))

            def proj_tc(tc4, tail=False):
                # oc-PAIR tiles with one fused ap-1024 drain each.  Tail
                # blocks (tc4 2,3) borrow the sps ring -- the score stream
                # is over, so its slots are free; early blocks follow
                # PJ_POOL (default: sh ring, whose slot just freed).
                for op2 in range(2):
                    if tail or PJ_POOL == 0:
                        pjt = ps_sps.tile([P, 2, 512], F32, tag="sps",
                                          name=f"pjs{tc4}{op2}")
                    elif PJ_POOL == 1:
                        pjt = ps_pj.tile([P, 2, 512], F32, tag="pj",
                                         name=f"pjs{tc4}{op2}")
                    else:
                        pjt = ps_sh.tile([P, 2, 512], F32, tag="sh",
                                         name=f"pjs{tc4}{op2}")
                    for o2 in range(2):
                        oc = op2 * 2 + o2
                        nc.tensor.matmul(
                            pjt[:, o2, :],
                            lhsT=wp[:, :, oc, :],
                            rhs=a_sb[:, :, tc4 * 512 : (tc4 + 1) * 512],
                            start=True, stop=True,
                            perf_mode=DR,
                        )
                    ot = outp.tile([P, 2, 512], BF16, name="otp")
                    if PJ_DRAIN == "act" or (PJ_DRAIN == "alt" and op2 == 0):
                        nc.scalar.activation(
                            ot, pjt, AF.Identity,
                            scale=float(1.0 / (SV * SW)))
                    else:
                        nc.vector.tensor_scalar(
                            ot, pjt, float(1.0 / (SV * SW)),
                            None, ALU.mult)
                    nc.sync.dma_start(
                        out_d[:, 2 * op2 : 2 * op2 + 2,
                              tc4 * 512 : (tc4 + 1) * 512], ot)

            run_attention()
            proj_tc(3, tail=True)
    nc.compile()
    return nc


_NC = None
_LAST_RESULTS = None


def _get_nc():
    global _NC
    if _NC is None:
        _NC = _build_nc()
    return _NC


def _fp8(a):
    return np.ascontiguousarray(a.astype(np.float32).astype(E4))


def kernel(x, mask, gn_gamma, gn_beta, qkv_w, qkv_b, proj_w, proj_b,
           _trace=False):
    del mask  # all-True per problem spec
    x = np.asarray(x, np.float32)
    gn_gamma = np.asarray(gn_gamma, np.float32)
    gn_beta = np.asarray(gn_beta, np.float32)
    qkv_w = np.asarray(qkv_w, np.float32)
    qkv_b = np.asarray(qkv_b, np.float32)
    proj_w = np.asarray(proj_w, np.float32)
    proj_b = np.asarray(proj_b, np.float32)

    # exact GroupNorm stats per batch (host, f32)
    xg = x.reshape(B, G, C // G, T)
    mu = xg.mean(axis=(2, 3))                      # [B, G]
    var = xg.var(axis=(2, 3))                      # [B, G]
    s_bg = 1.0 / np.sqrt(var + EPS)                # [B, G]
    s_bc = np.repeat(s_bg, C // G, axis=1) * gn_gamma[None, :]      # [B, C]
    off_bc = gn_beta[None, :] - np.repeat(mu * s_bg, C // G, axis=1) \
        * gn_gamma[None, :]                        # [B, C]

    in_maps = []
    v_bias_term = {}
    for core in range(N_CORES):
        b, hh = core // 2, core % 2
        heads = [hh * HL + i for i in range(HL)]
        # column order for q/k: [head][ch]; mc blocks = head pairs
        q_rows = np.concatenate(
            [np.arange(h * 192, h * 192 + 64) for h in heads])
        k_rows = q_rows + 64
        v_rows = np.concatenate([np.arange(h * 192 + 128, h * 192 + 192)
                                 for h in heads])

        s = s_bc[b]                                # [C]
        off = off_bc[b]                            # [C]

        wq = qkv_w[q_rows] * s[None, :]            # [256, 512]
        wk = qkv_w[k_rows] * s[None, :]
        wv_ = qkv_w[v_rows] * s[None, :]
        # wqk dram layout [p(c%128), kc(c//128), mc, m(128)]
        wqk_m = np.concatenate([wq, wk], 0)        # [512(m), 512(c)]
        wqk_t = (wqk_m.T.reshape(4, P, 4, P)
                 .transpose(1, 0, 2, 3))           # [p, kc, mc, m]
        wqk_t = wqk_t * SW
        wv_t = wv_.T.reshape(4, P, 2 * P).transpose(1, 0, 2) * SW
        # proj columns for this half, reordered to head-band x ch
        wp_cols = proj_w[:, [hh * 256 + i for i in range(256)]]  # [512, 256]
        # a_sb rows: [hd%2 band (64), hd//2 ktile]: channel (hd, ch) sits at
        # row 64*(hd%2)+ch of ktile hd//2 -> input index hd*64+ch
        perm = np.array([(kt * 2 + band) * 64 + ch
                         for kt in range(2) for band in range(2)
                         for ch in range(64)])
        # rows of wp lhsT tile [p, kt, oc, m]: p = 64*band+ch
        wp_in = wp_cols[:, perm]                   # [512 out, 256 perm-in]
        wp_t = (wp_in.T.reshape(2, P, 4, P)
                .transpose(1, 0, 2, 3)) * SW       # [p, kt, oc, m]

        cq = (qkv_w[q_rows] @ off + qkv_b[q_rows]) * SCALE * SQ
        ck = (qkv_w[k_rows] @ off + qkv_b[k_rows]) * SCALE * SQ
        cqk = np.stack([cq[:P], cq[P:], ck[:P], ck[P:]], axis=1)  # [128, 4]

        x_t = x[b].reshape(4, P, T).transpose(1, 0, 2)

        in_maps.append(dict(
            x=_fp8(x_t),
            wqk=_fp8(wqk_t),
            wv=_fp8(wv_t),
            wp=_fp8(wp_t),
            cqk=np.ascontiguousarray(cqk, dtype=np.float32),
        ))
        # v bias + GN-offset contribution through v, exact on host:
        cv = qkv_w[v_rows] @ off + qkv_b[v_rows]   # [256]
        v_bias_term[core] = proj_w[:, hh * 256 : hh * 256 + 256] @ cv  # [512]

    nc = _get_nc()
    res = run_bass_kernel_spmd(nc, in_maps, core_ids=list(range(N_CORES)),
                               trace=_trace)
    global _LAST_RESULTS
    _LAST_RESULTS = res
    out = np.empty((B, C, T), np.float32)
    for b in range(B):
        r0 = res.results[2 * b]["out"].astype(np.float32)
        r1 = res.results[2 * b + 1]["out"].astype(np.float32)
        const = (v_bias_term[2 * b] + v_bias_term[2 * b + 1]
                 + proj_b)[:, None]
        out[b] = (x[b]
                  + r0.transpose(1, 0, 2).reshape(C, T)
                  + r1.transpose(1, 0, 2).reshape(C, T)
                  + const)
    return out


# revision 27
# speedup vs baseline: 1.0252x; 1.0252x over previous
"""AttentionBlock (GroupNorm -> qkv -> softmax attention -> proj + residual)
for Trainium2, 8 NeuronCores, fp8 DoubleRow edition.

Sharding: core = (batch b, head-half hh): each core handles 1 of 4 batches
and 4 of 8 heads, computing a partial projection output; the host sums the
two partials per batch and adds the residual x and proj_b.

Device-side structure (per core):
 - GroupNorm is folded into the weights on the HOST: h = s*x + off with
   per-(batch,channel) s/off from exact f32 stats, so W' = W*diag(s) (fp8)
   and per-out-channel biases ride the PSUM->SBUF drains.  x ships as fp8.
 - ALL matmuls (qkv/scores/av/proj) run in fp8e4 DoubleRow perf mode
   (0.5 cycles/row).  The score matmuls have only a 64-deep contraction
   (per-head channels); DoubleRow needs a k-tile PAIR, so q/k live in
   [128, 2(kt), 4(mc), T] tiles whose kt=1 plane is zero: lhsT/rhs APs
   [64, 2, m] contract over (64 ch + 64 zeros) -- numerically exact, and
   the cost halves.
 - exp(scores) is split between the ACT engine (native Exp) and the DVE
   (a custom quartic c2*(c0*x+c1)^4 DVE op registered at import time).
 - softmax normalization: rowsums come free via a ones-column in v^T; per
   unit ONE fused reciprocal [1,2,512] (DVE), ONE partition_broadcast
   (GPSIMD) and ONE multiply [64,2,512] (DVE) cover both t-halves.

The mask input is all-True per the problem spec, a numeric no-op.  q/k/GN
biases are folded exactly; v/proj biases are added exactly on the host.
"""

import os
import numpy as np
import ml_dtypes

import concourse.bass as bass
import concourse.tile as tile
from concourse import bacc, mybir, library_config
from concourse.bass_utils import run_bass_kernel_spmd

F32 = mybir.dt.float32
BF16 = mybir.dt.bfloat16
FP8 = mybir.dt.float8e4
AF = mybir.ActivationFunctionType
ALU = mybir.AluOpType
DR = mybir.MatmulPerfMode.DoubleRow
E4 = ml_dtypes.float8_e4m3

B, C, T, H = 4, 512, 2048, 8
CH = 64
G = 32
EPS = 1e-5
HL = 4                 # heads per core
P = 128
TH = T // 2            # 1024, t-half handled per (hd, th)
N_CORES = 8

# fp8 range scales
SW = 64.0              # weight upscale for fp8 (qkv + proj weights)
SQ = 4.0               # q/k sbuf upscale (on top of 1/sqrt(sqrt(ch)))
SV = 64.0              # v sbuf upscale (=SW so vt drain is a plain copy)
GAMMA = 1.0 / (SQ * SQ)  # descale applied inside exp
SCALE = 1.0 / np.sqrt(np.sqrt(CH))

# quartic exp approximation constants (minimax on [-1.7, 1.7])
QC0, QC1, QC2 = 0.24274105, 1.02873227, 1.04374374

# exp engine split: chunk i of 16 per (hd, th) goes to ACT if pattern bit set
EXP_ACT_FRAC = float(os.environ.get("EXP_ACT_FRAC", "0.59"))
# drain engine choices ("act" or "dve")
QK_DRAIN = os.environ.get("QK_DRAIN", "alt")
VT_DRAIN = os.environ.get("VT_DRAIN", "act")
PJ_DRAIN = os.environ.get("PJ_DRAIN", "alt")
VT_POS = os.environ.get("VT_POS", "spread")
WPOOL = int(os.environ.get("WPOOL", "12"))
VT_SLOTS = tuple(int(v) for v in os.environ.get("VT_SLOTS", "1,3,5,7").split(","))
WARMUP = int(os.environ.get("WARMUP", "12"))
NORM_LAG = int(os.environ.get("NORM_LAG", "0"))
SPLIT_LAST = int(os.environ.get("SPLIT_LAST", "0"))
OUTP = int(os.environ.get("OUTP", "6"))
# PJ_POOL: where proj psum tiles come from. 0 = borrow the sps ring
# (stalls the score stream ~1us per burst); 2 = borrow the sh (avs) ring
# (proj naturally waits for the freshly-freed avs slot).  1 = dedicated
# pool, only with SPS_BUFS=2 — measured much worse, keep for reference.
PJ_POOL = int(os.environ.get("PJ_POOL", "2"))
SPS_BUFS = int(os.environ.get("SPS_BUFS", "2" if PJ_POOL == 1 else "3"))

# ---- custom DVE op: EXP4_ANT = c2*(c0*x+c1)^4 ------------------------------
from concourse import dve_ops as _dops
from concourse.dve_spec import Spec as _Spec, Src0 as _Src0, C0 as _C0, \
    C1 as _C1, C2 as _C2, sq as _sq, lower as _lower
from concourse.dve_uop import DveOpSpec as _DveOpSpec


def _exp4_ref(in0, in1, c0, c1, c2):
    y = np.square(np.square(in0.astype(np.float32) * c0 + c1)) * c2
    return y.astype(np.float32)


def _register_exp4():
    for op in _dops.OPS:
        if op.name == "EXP4_ANT":
            return op
    spec = _Spec(body=_sq(_sq(_Src0 * _C0 + _C1)) * _C2, reference=_exp4_ref)
    shas = {}
    for ver in ("v3", "v4"):
        s = _DveOpSpec(name="EXP4_ANT", opcode=0, uops=_lower(spec, ver=ver),
                       rd1_en=False)
        shas[ver] = s.sha(ver)
    op = _dops.DveOp("EXP4_ANT", spec, subdim=False, uops_sha=shas)
    _dops.OPS.append(op)
    _dops.CUSTOM_DVE_SPECS[op.name] = spec
    _dops._SUB_OPCODE_FOR_NAME[op.name] = (
        max(_dops._SUB_OPCODE_FOR_NAME.values()) + 1)
    return op


EXP4 = _register_exp4()


EXP_ACT_EARLY = float(os.environ.get("EXP_ACT_EARLY", "0.50"))
EXP_EARLY_CHUNKS = int(os.environ.get("EXP_EARLY_CHUNKS", "16"))
EXP_ACT_LATE = float(os.environ.get("EXP_ACT_LATE", "0.55"))
EXP_LATE_CHUNKS = int(os.environ.get("EXP_LATE_CHUNKS", "16"))


def _exp_engine_pattern():
    """One entry per exp chunk (128 total): True -> ACT, False -> DVE.
    Early chunks lean DVE (ACT busy with qkv drains); late chunks move
    toward 50/50 so both engines finish the last unit together."""
    total_act = EXP_ACT_FRAC * 128
    early_act = EXP_ACT_EARLY * EXP_EARLY_CHUNKS
    late_act = EXP_ACT_LATE * EXP_LATE_CHUNKS
    mid = 128 - EXP_EARLY_CHUNKS - EXP_LATE_CHUNKS
    mid_frac = (total_act - early_act - late_act) / mid
    pat = []
    acc = 0.0
    for i in range(128):
        if i < EXP_EARLY_CHUNKS:
            f = EXP_ACT_EARLY
        elif i >= 128 - EXP_LATE_CHUNKS:
            f = EXP_ACT_LATE
        else:
            f = mid_frac
        acc += f
        if acc >= 1.0 - 1e-9:
            acc -= 1.0
            pat.append(True)
        else:
            pat.append(False)
    return pat


def _build_nc():
    nc = bacc.Bacc(
        "TRN2",
        target_bir_lowering=False,
        debug=False,
        enable_asserts=False,
        num_devices=N_CORES,
    )
    x_d = nc.dram_tensor("x", [P, 4, T], FP8, kind="ExternalInput").ap()
    wqk_d = nc.dram_tensor("wqk", [P, 4, 4, P], FP8, kind="ExternalInput").ap()
    wv_d = nc.dram_tensor("wv", [P, 4, 2 * P], FP8, kind="ExternalInput").ap()
    wp_d = nc.dram_tensor("wp", [P, 2, 4, P], FP8, kind="ExternalInput").ap()
    cqk_d = nc.dram_tensor("cqk", [P, 4], F32, kind="ExternalInput").ap()
    out_d = nc.dram_tensor("out", [P, 4, T], BF16, kind="ExternalOutput").ap()

    pat = _exp_engine_pattern()

    with tile.TileContext(nc) as tc:
        with (
            tc.tile_pool(name="consts", bufs=1) as consts,
            tc.tile_pool(name="xp", bufs=1) as xp,
            tc.tile_pool(name="qkp", bufs=1) as qkp,
            tc.tile_pool(name="vtp", bufs=1) as vtp,
            tc.tile_pool(name="ap", bufs=1) as apool,
            tc.tile_pool(name="wpool", bufs=WPOOL) as wpool,
            tc.tile_pool(name="rhop", bufs=3) as rhop,
            tc.tile_pool(name="repp", bufs=3) as repp,
            tc.tile_pool(name="outp", bufs=OUTP) as outp,
            tc.tile_pool(name="ps_sps", bufs=SPS_BUFS, space="PSUM") as ps_sps,
            tc.tile_pool(name="ps_pj", bufs=1, space="PSUM") as ps_pj,
            tc.tile_pool(name="ps_sh", bufs=1, space="PSUM") as ps_sh,
        ):
            nc.gpsimd.load_library(library_config.attn)

            # ---- DMA in ----
            # DMA_GP=1: input DMAs issue from the GPSIMD queue (Pool DMA
            # config is 25ns/instr vs SP's 565) and x splits into quarters
            # so the 16 parallel DMA engines land it sooner.
            dma_eng = nc.gpsimd if int(os.environ.get("DMA_GP", "0")) \
                else nc.sync
            x_sb = xp.tile([P, 4, T], FP8)
            dma_eng.dma_start(x_sb[:, :, 0:TH], x_d[:, :, 0:TH])
            wqk = consts.tile([P, 4, 4, P], FP8)
            dma_eng.dma_start(wqk, wqk_d)
            dma_eng.dma_start(x_sb[:, :, TH:T], x_d[:, :, TH:T])
            cqk = consts.tile([P, 4], F32)
            dma_eng.dma_start(cqk, cqk_d)
            wv = consts.tile([P, 4, 2 * P], FP8)
            dma_eng.dma_start(wv, wv_d)
            wp = consts.tile([P, 2, 4, P], FP8)
            dma_eng.dma_start(wp, wp_d)

            # PE p-state warmup while input DMAs land: dummy matmuls on a
            # const tile keep the PE continuously busy so real matmuls start
            # at full clock.
            warm = consts.tile([P, P], FP8)
            nc.vector.memset(warm, 0.0)
            warm2 = consts.tile([P, 512], FP8)
            nc.vector.memset(warm2, 0.0)
            warm_ps = ps_sps.tile([P, 512], F32, tag="sps", name="warm")
            for _ in range(WARMUP):
                nc.tensor.matmul(warm_ps[:, 0:128], lhsT=warm,
                                 rhs=warm2[:, 0:128], start=True, stop=True)

            # ---- qk matmuls + drains ----
            # qk_sb: [128, 2 (kt), 4 (mc), T] fp8.  kt=0 holds q/k data
            # (mc 0: q heads 0/1, 1: q heads 2/3, 2: k heads 0/1, 3: k
            # heads 2/3; head parity is the 64-partition band), kt=1 is
            # ZERO so score matmuls can run DoubleRow with APs
            # [64, 2(kt), m] -- contraction (64 ch + 64 zeros).
            qk_sb = qkp.tile([P, 2, 4, T], FP8)
            # zero the kt=1 planes on the (otherwise idle) GPSIMD engine,
            # in first-use order: k_a, q_a, k_b, q_b
            for mc in (2, 0, 3, 1):
                nc.gpsimd.memset(qk_sb[:, 1, mc, :], 0.0)

            def qk_group(mc, tc2):
                # fused [128, 1024] tile (two tc4 halves) in the sps pool
                qkt = ps_sps.tile([P, 2, 512], F32, tag="sps",
                                  name=f"qk{mc}{tc2}")
                for t2 in range(2):
                    tc4 = tc2 * 2 + t2
                    for kcp in range(2):
                        nc.tensor.matmul(
                            qkt[:, t2, :],
                            lhsT=wqk[:, 2 * kcp : 2 * kcp + 2, mc, :],
                            rhs=x_sb[:, 2 * kcp : 2 * kcp + 2,
                                     tc4 * 512 : (tc4 + 1) * 512],
                            start=(kcp == 0), stop=(kcp == 1),
                            perf_mode=DR,
                        )
                dst = qk_sb[:, 0, mc, tc2 * 1024 : (tc2 + 1) * 1024]
                if QK_DRAIN == "act" or (QK_DRAIN == "alt" and mc in (0, 1)) \
                        or (QK_DRAIN == "alt2" and mc in (2, 3)):
                    nc.scalar.activation(
                        dst,
                        qkt.rearrange("p a b -> p (a b)"),
                        AF.Identity,
                        bias=cqk[:, mc : mc + 1],
                        scale=float(SCALE * SQ / SW),
                    )
                else:
                    nc.vector.tensor_scalar(
                        dst,
                        qkt.rearrange("p a b -> p (a b)"),
                        float(SCALE * SQ / SW),
                        cqk[:, mc : mc + 1],
                        ALU.mult, ALU.add,
                    )

            # ---- vt matmuls + drains ----
            # vt_sb: [128 (s%128), 16 (sc), 4 (hd), 128] fp8; cols 64..127
            # are ONES so av rows 64..127 all come out as the rowsum -- a
            # 64-way replicated rowsum that feeds reciprocal directly (no
            # partition_broadcast needed).
            vt_sb = vtp.tile([P, 16, HL, 2 * CH], FP8)
            nc.gpsimd.memset(vt_sb[:, :, :, CH : 2 * CH], 1.0)

            def vt_group4(g):
                # fused tile: 4 sc chunks (= scp pair 2g, 2g+1)
                vtt = ps_sps.tile([P, 4, 2 * P], F32, tag="sps",
                                  name=f"vt{g}")
                for s4 in range(4):
                    sc = g * 4 + s4
                    for kcp in range(2):
                        nc.tensor.matmul(
                            vtt[:, s4, :],
                            lhsT=x_sb[:, 2 * kcp : 2 * kcp + 2,
                                      sc * P : (sc + 1) * P],
                            rhs=wv[:, 2 * kcp : 2 * kcp + 2, :],
                            start=(kcp == 0), stop=(kcp == 1),
                            perf_mode=DR,
                        )
                if VT_DRAIN == "act" or (VT_DRAIN == "alt" and g % 2 == 0):
                    nc.scalar.activation(
                        vt_sb[:, 4 * g : 4 * g + 4, :, 0:CH],
                        vtt.rearrange("p a (h c) -> p a h c", h=HL),
                        AF.Identity,
                    )
                else:
                    nc.vector.tensor_copy(
                        vt_sb[:, 4 * g : 4 * g + 4, :, 0:CH],
                        vtt.rearrange("p a (h c) -> p a h c", h=HL),
                    )

            # qk for the FIRST chunks only (k_a/q_a tc2=0); the remaining
            # 6 qk groups and the 4 vt groups are injected INTO the
            # attention stream (QK_STREAM=1) so the first score matmul
            # fires as soon as the first x half + wqk land, instead of
            # after all 32 qk matmuls.
            QK_STREAM = int(os.environ.get("QK_STREAM", "1"))
            qk_group(2, 0)                 # k_a s 0..1024
            qk_group(0, 0)                 # q_a t 0..1024
            if QK_STREAM:
                stream_extras = {
                    0: lambda: qk_group(2, 1),   # k_a s 1024.. (need g4)
                    1: lambda: vt_group4(0),     # sc 0-3      (need g5)
                    2: lambda: qk_group(3, 0),   # k_b         (need g16)
                    3: lambda: qk_group(1, 0),   # q_b         (need g16)
                    4: lambda: vt_group4(1),     # sc 4-7      (need g7)
                    5: lambda: qk_group(0, 1),   # q_a th1     (need g32)
                    6: lambda: vt_group4(2),     # sc 8-11     (need g9)
                    7: lambda: vt_group4(3),     # sc 12-15    (need g11)
                    8: lambda: qk_group(3, 1),   # k_b s 1024..(need g20)
                    9: lambda: qk_group(1, 1),   # q_b th1     (need g48)
                }
            else:
                qk_group(2, 1)
                qk_group(0, 1)
                for tc2 in range(2):
                    qk_group(3, tc2)
                    qk_group(1, tc2)
                stream_extras = {}
                if VT_POS == "pre":
                    for g in range(4):
                        vt_group4(g)

            # ---- attention ----
            a_sb = apool.tile([P, 2, T], FP8)

            # attention as a software-pipelined chunk stream: av matmuls
            # lag the scores/exp stream by AV_LAG chunk-pairs so PE never
            # waits on the previous unit's last exp at unit boundaries.
            AV_LAG = int(os.environ.get("AV_LAG", "5"))
            units = [(hd, th) for th in range(2) for hd in range(HL)]
            state = {}   # u -> dict(avs, w_ts)
            exp_ctr = [0]

            def unit_geom(u):
                hd, th = units[u]
                b0 = 64 * (hd % 2)
                q_mc = 0 if hd < 2 else 1
                k_mc = 2 if hd < 2 else 3
                return hd, th, b0, q_mc, k_mc

            def emit_chunk(u, scp):
                hd, th, b0, q_mc, k_mc = unit_geom(u)
                toff = th * TH
                if scp == 0:
                    state[u] = dict(
                        avs=ps_sh.tile([P, 2, 512], F32, tag="sh",
                                       name=f"av{hd}{th}"),
                        w_ts={})
                w_t = wpool.tile([P, 2, TH], FP8, name="wt")
                state[u]["w_ts"][scp] = w_t
                split = u >= len(units) - SPLIT_LAST
                for j in range(2):
                    sc = scp * 2 + j
                    sps = ps_sps.tile([P, TH], F32, tag="sps", name="sps")
                    for tq in range(2):
                        nc.tensor.matmul(
                            sps[:, tq * 512 : (tq + 1) * 512],
                            lhsT=qk_sb[b0 : b0 + CH, :, k_mc,
                                       sc * P : (sc + 1) * P],
                            rhs=qk_sb[b0 : b0 + CH, :, q_mc,
                                      toff + tq * 512 : toff + (tq + 1) * 512],
                            start=True, stop=True,
                            perf_mode=DR,
                        )
                    if split:
                        # tail units: halve each chunk across BOTH engines so
                        # the slot frees sooner and av-tq halves unblock early
                        nc.scalar.activation(
                            w_t[:, j, 0:512], sps[:, 0:512], AF.Exp,
                            scale=float(GAMMA))
                        nc.vector._custom_dve(
                            EXP4, out=w_t[:, j, 512:TH], in0=sps[:, 512:TH],
                            s0=float(QC0 * GAMMA), s1=float(QC1),
                            imm2=float(QC2))
                    elif pat[exp_ctr[0]]:
                        nc.scalar.activation(
                            w_t[:, j, :], sps, AF.Exp, scale=float(GAMMA))
                    else:
                        nc.vector._custom_dve(
                            EXP4, out=w_t[:, j, :], in0=sps,
                            s0=float(QC0 * GAMMA), s1=float(QC1),
                            imm2=float(QC2))
                    exp_ctr[0] += 1

            def emit_av(u, scp):
                hd, th, b0, q_mc, k_mc = unit_geom(u)
                avs = state[u]["avs"]
                w_t = state[u]["w_ts"].pop(scp)
                for tq in range(2):
                    nc.tensor.matmul(
                        avs[:, tq, :],
                        lhsT=vt_sb[:, 2 * scp : 2 * scp + 2, hd, :],
                        rhs=w_t[:, :, tq * 512 : (tq + 1) * 512],
                        start=(scp == 0), stop=(scp == 7),
                        perf_mode=DR,
                    )

            def emit_normalize(u, between=None):
                hd, th, b0, q_mc, k_mc = unit_geom(u)
                toff = th * TH
                avs = state[u]["avs"]
                if between is None:
                    # fused across both tq halves: reciprocal of the 64
                    # replicated rowsum rows IS the broadcast recip; then
                    # one multiply [64,2,512]
                    rep = repp.tile([CH, 2, 512], F32, name="rep")
                    nc.vector.reciprocal(rep, avs[CH : 2 * CH, :, :])
                    nc.vector.tensor_tensor(
                        a_sb[CH * (hd % 2) : CH * (hd % 2) + CH, hd // 2,
                             toff : toff + TH],
                        avs[0:CH, :, :], rep, ALU.mult,
                    )
                else:
                    # last unit: split per tq so proj_tc(2) can interleave
                    for tq in range(2):
                        rep = repp.tile([CH, 512], F32, name="rep")
                        nc.vector.reciprocal(rep, avs[CH : 2 * CH, tq, :])
                        nc.vector.tensor_tensor(
                            a_sb[CH * (hd % 2) : CH * (hd % 2) + CH, hd // 2,
                                 toff + tq * 512 : toff + (tq + 1) * 512],
                            avs[0:CH, tq, :], rep, ALU.mult,
                        )
                        if tq == 0:
                            between()
                del state[u]

            def run_attention(extra=()):
                stream = [(u, scp) for u in range(len(units))
                          for scp in range(8)]
                norm_q = []   # units whose avs are done, normalize deferred

                def pop_norm():
                    lu = norm_q.pop(0)
                    emit_normalize(lu)
                    if lu == 3:           # last th0 unit done
                        proj_tc(0)
                    elif lu == 5:
                        proj_tc(1)

                for g, (u, scp) in enumerate(stream):
                    emit_chunk(u, scp)
                    if QK_STREAM:
                        if g in stream_extras:
                            stream_extras[g]()
                    elif VT_POS == "stream" and g < 4:
                        vt_group4(g)
                    elif VT_POS == "spread" and g in VT_SLOTS:
                        vt_group4(VT_SLOTS.index(g))
                    lag = g - AV_LAG
                    if lag >= 0:
                        lu, lscp = stream[lag]
                        emit_av(lu, lscp)
                        if lscp == 7:
                            norm_q.append(lu)
                    if norm_q:
                        lu = norm_q[0]
                        close_g = (lu * 8 + 7) + AV_LAG  # g when avs closed
                        if g >= close_g + NORM_LAG:
                            pop_norm()
                for lu, lscp in stream[-AV_LAG:]:
                    emit_av(lu, lscp)
                    if lscp == 7:
                        norm_q.append(lu)
                while len(norm_q) > 1:
                    pop_norm()
                emit_normalize(norm_q.pop(0),
                               between=lambda: proj_tc(2, tail=True))

            def proj_tc(tc4, tail=False):
                # oc-PAIR tiles with one fused ap-1024 drain each.  Tail
                # blocks (tc4 2,3) borrow the sps ring -- the score stream
                # is over, so its slots are free; early blocks follow
                # PJ_POOL (default: sh ring, whose slot just freed).
                for op2 in range(2):
                    if tail or PJ_POOL == 0:
                        pjt = ps_sps.tile([P, 2, 512], F32, tag="sps",
                                          name=f"pjs{tc4}{op2}")
                    elif PJ_POOL == 1:
                        pjt = ps_pj.tile([P, 2, 512], F32, tag="pj",
                                         name=f"pjs{tc4}{op2}")
                    else:
                        pjt = ps_sh.tile([P, 2, 512], F32, tag="sh",
                                         name=f"pjs{tc4}{op2}")
                    for o2 in range(2):
                        oc = op2 * 2 + o2
                        nc.tensor.matmul(
                            pjt[:, o2, :],
                            lhsT=wp[:, :, oc, :],
                            rhs=a_sb[:, :, tc4 * 512 : (tc4 + 1) * 512],
                            start=True, stop=True,
                            perf_mode=DR,
                        )
                    ot = outp.tile([P, 2, 512], BF16, name="otp")
                    if PJ_DRAIN == "act" or (PJ_DRAIN == "alt" and op2 == 0):
                        nc.scalar.activation(
                            ot, pjt, AF.Identity,
                            scale=float(1.0 / (SV * SW)))
                    else:
                        nc.vector.tensor_scalar(
                            ot, pjt, float(1.0 / (SV * SW)),
                            None, ALU.mult)
                    nc.sync.dma_start(
                        out_d[:, 2 * op2 : 2 * op2 + 2,
                              tc4 * 512 : (tc4 + 1) * 512], ot)

            run_attention()
            proj_tc(3, tail=True)
    nc.compile()
    return nc


_NC = None
_LAST_RESULTS = None


def _get_nc():
    global _NC
    if _NC is None:
        _NC = _build_nc()
    return _NC


def _fp8(a):
    return np.ascontiguousarray(a.astype(np.float32).astype(E4))


def kernel(x, mask, gn_gamma, gn_beta, qkv_w, qkv_b, proj_w, proj_b,
           _trace=False):
    del mask  # all-True per problem spec
    x = np.asarray(x, np.float32)
    gn_gamma = np.asarray(gn_gamma, np.float32)
    gn_beta = np.asarray(gn_beta, np.float32)
    qkv_w = np.asarray(qkv_w, np.float32)
    qkv_b = np.asarray(qkv_b, np.float32)
    proj_w = np.asarray(proj_w, np.float32)
    proj_b = np.asarray(proj_b, np.float32)

    # exact GroupNorm stats per batch (host, f32)
    xg = x.reshape(B, G, C // G, T)
    mu = xg.mean(axis=(2, 3))                      # [B, G]
    var = xg.var(axis=(2, 3))                      # [B, G]
    s_bg = 1.0 / np.sqrt(var + EPS)                # [B, G]
    s_bc = np.repeat(s_bg, C // G, axis=1) * gn_gamma[None, :]      # [B, C]
    off_bc = gn_beta[None, :] - np.repeat(mu * s_bg, C // G, axis=1) \
        * gn_gamma[None, :]                        # [B, C]

    in_maps = []
    v_bias_term = {}
    for core in range(N_CORES):
        b, hh = core // 2, core % 2
        heads = [hh * HL + i for i in range(HL)]
        # column order for q/k: [head][ch]; mc blocks = head pairs
        q_rows = np.concatenate(
            [np.arange(h * 192, h * 192 + 64) for h in heads])
        k_rows = q_rows + 64
        v_rows = np.concatenate([np.arange(h * 192 + 128, h * 192 + 192)
                                 for h in heads])

        s = s_bc[b]                                # [C]
        off = off_bc[b]                            # [C]

        wq = qkv_w[q_rows] * s[None, :]            # [256, 512]
        wk = qkv_w[k_rows] * s[None, :]
        wv_ = qkv_w[v_rows] * s[None, :]
        # wqk dram layout [p(c%128), kc(c//128), mc, m(128)]
        wqk_m = np.concatenate([wq, wk], 0)        # [512(m), 512(c)]
        wqk_t = (wqk_m.T.reshape(4, P, 4, P)
                 .transpose(1, 0, 2, 3))           # [p, kc, mc, m]
        wqk_t = wqk_t * SW
        wv_t = wv_.T.reshape(4, P, 2 * P).transpose(1, 0, 2) * SW
        # proj columns for this half, reordered to head-band x ch
        wp_cols = proj_w[:, [hh * 256 + i for i in range(256)]]  # [512, 256]
        # a_sb rows: [hd%2 band (64), hd//2 ktile]: channel (hd, ch) sits at
        # row 64*(hd%2)+ch of ktile hd//2 -> input index hd*64+ch
        perm = np.array([(kt * 2 + band) * 64 + ch
                         for kt in range(2) for band in range(2)
                         for ch in range(64)])
        # rows of wp lhsT tile [p, kt, oc, m]: p = 64*band+ch
        wp_in = wp_cols[:, perm]                   # [512 out, 256 perm-in]
        wp_t = (wp_in.T.reshape(2, P, 4, P)
                .transpose(1, 0, 2, 3)) * SW       # [p, kt, oc, m]

        cq = (qkv_w[q_rows] @ off + qkv_b[q_rows]) * SCALE * SQ
        ck = (qkv_w[k_rows] @ off + qkv_b[k_rows]) * SCALE * SQ
        cqk = np.stack([cq[:P], cq[P:], ck[:P], ck[P:]], axis=1)  # [128, 4]

        x_t = x[b].reshape(4, P, T).transpose(1, 0, 2)

        in_maps.append(dict(
            x=_fp8(x_t),
            wqk=_fp8(wqk_t),
            wv=_fp8(wv_t),
            wp=_fp8(wp_t),
            cqk=np.ascontiguousarray(cqk, dtype=np.float32),
        ))
        # v bias + GN-offset contribution through v, exact on host:
        cv = qkv_w[v_rows] @ off + qkv_b[v_rows]   # [256]
        v_bias_term[core] = proj_w[:, hh * 256 : hh * 256 + 256] @ cv  # [512]

    nc = _get_nc()
    res = run_bass_kernel_spmd(nc, in_maps, core_ids=list(range(N_CORES)),
                               trace=_trace)
    global _LAST_RESULTS
    _LAST_RESULTS = res
    out = np.empty((B, C, T), np.float32)
    for b in range(B):
        r0 = res.results[2 * b]["out"].astype(np.float32)
        r1 = res.results[2 * b + 1]["out"].astype(np.float32)
        const = (v_bias_term[2 * b] + v_bias_term[2 * b + 1]
                 + proj_b)[:, None]
        out[b] = (x[b]
                  + r0.transpose(1, 0, 2).reshape(C, T)
                  + r1.transpose(1, 0, 2).reshape(C, T)
                  + const)
    return out


# revision 39
# speedup vs baseline: 1.0657x; 1.0394x over previous
"""AttentionBlock (GroupNorm -> qkv -> softmax attention -> proj + residual)
for Trainium2, 8 NeuronCores, fp8 DoubleRow edition.

Sharding: core = (batch b, head-half hh): each core handles 1 of 4 batches
and 4 of 8 heads, computing a partial projection output; the host sums the
two partials per batch and adds the residual x and proj_b.

Device-side structure (per core):
 - GroupNorm is folded into the weights on the HOST: h = s*x + off with
   per-(batch,channel) s/off from exact f32 stats, so W' = W*diag(s) (fp8)
   and per-out-channel biases ride the PSUM->SBUF drains.  x ships as fp8.
 - ALL matmuls (qkv/scores/av/proj) run in fp8e4 DoubleRow perf mode
   (0.5 cycles/row).  The score matmuls have only a 64-deep contraction
   (per-head channels); DoubleRow needs a k-tile PAIR, so q/k live in
   [128, 2(kt), 4(mc), T] tiles whose kt=1 plane is zero: lhsT/rhs APs
   [64, 2, m] contract over (64 ch + 64 zeros) -- numerically exact, and
   the cost halves.
 - exp(scores) is split between the ACT engine (native Exp) and the DVE
   (a custom quartic c2*(c0*x+c1)^4 DVE op registered at import time).
 - softmax normalization: rowsums come free via a ones-column in v^T; per
   unit ONE fused reciprocal [1,2,512] (DVE), ONE partition_broadcast
   (GPSIMD) and ONE multiply [64,2,512] (DVE) cover both t-halves.

The mask input is all-True per the problem spec, a numeric no-op.  q/k/GN
biases are folded exactly; v/proj biases are added exactly on the host.
"""

import os
import numpy as np
import ml_dtypes

import concourse.bass as bass
import concourse.tile as tile
from concourse import bacc, mybir, library_config
from concourse.bass_utils import run_bass_kernel_spmd

F32 = mybir.dt.float32
BF16 = mybir.dt.bfloat16
FP8 = mybir.dt.float8e4
AF = mybir.ActivationFunctionType
ALU = mybir.AluOpType
DR = mybir.MatmulPerfMode.DoubleRow
E4 = ml_dtypes.float8_e4m3

B, C, T, H = 4, 512, 2048, 8
CH = 64
G = 32
EPS = 1e-5
HL = 4                 # heads per core
P = 128
TH = T // 2            # 1024, t-half handled per (hd, th)
N_CORES = 8

# fp8 range scales
SW = 64.0              # weight upscale for fp8 (qkv + proj weights)
SQ = 4.0               # q/k sbuf upscale (on top of 1/sqrt(sqrt(ch)))
SV = 64.0              # v sbuf upscale (=SW so vt drain is a plain copy)
GAMMA = 1.0 / (SQ * SQ)  # descale applied inside exp
SCALE = 1.0 / np.sqrt(np.sqrt(CH))

# quartic exp approximation constants (minimax on [-1.7, 1.7])
QC0, QC1, QC2 = 0.24274105, 1.02873227, 1.04374374

# exp engine split: chunk i of 16 per (hd, th) goes to ACT if pattern bit set
EXP_ACT_FRAC = float(os.environ.get("EXP_ACT_FRAC", "0.59"))
# drain engine choices ("act" or "dve")
QK_DRAIN = os.environ.get("QK_DRAIN", "alt")
VT_DRAIN = os.environ.get("VT_DRAIN", "act")
PJ_DRAIN = os.environ.get("PJ_DRAIN", "alt")
# tail proj blocks drain on ACT only, so the last unit's normalize chain
# never queues behind a proj drain on the in-order DVE
PJT_TAIL_ACT = int(os.environ.get("PJT_TAIL_ACT", "1"))

# split the very last proj block's drains into ap-512 halves on both
# engines so the final out-DMA starts sooner
PJT_SPLIT_LAST = int(os.environ.get("PJT_SPLIT_LAST", "0"))
# final proj block: first drain on DVE (frees before ACT at the tail)
PJT3_DVE = int(os.environ.get("PJT3_DVE", "1"))
VT_POS = os.environ.get("VT_POS", "spread")
WPOOL = int(os.environ.get("WPOOL", "12"))
VT_SLOTS = tuple(int(v) for v in os.environ.get("VT_SLOTS", "1,3,5,7").split(","))
WARMUP = int(os.environ.get("WARMUP", "12"))
NORM_LAG = int(os.environ.get("NORM_LAG", "0"))
SPLIT_LAST = int(os.environ.get("SPLIT_LAST", "0"))
OUTP = int(os.environ.get("OUTP", "6"))
# PJ_POOL: where proj psum tiles come from. 0 = borrow the sps ring
# (stalls the score stream ~1us per burst); 2 = borrow the sh (avs) ring
# (proj naturally waits for the freshly-freed avs slot).  1 = dedicated
# pool, only with SPS_BUFS=2 — measured much worse, keep for reference.
PJ_POOL = int(os.environ.get("PJ_POOL", "2"))
SPS_BUFS = int(os.environ.get("SPS_BUFS", "2" if PJ_POOL == 1 else "3"))

# ---- custom DVE op: EXP4_ANT = c2*(c0*x+c1)^4 ------------------------------
from concourse import dve_ops as _dops
from concourse.dve_spec import Spec as _Spec, Src0 as _Src0, C0 as _C0, \
    C1 as _C1, C2 as _C2, sq as _sq, lower as _lower
from concourse.dve_uop import DveOpSpec as _DveOpSpec


def _exp4_ref(in0, in1, c0, c1, c2):
    y = np.square(np.square(in0.astype(np.float32) * c0 + c1)) * c2
    return y.astype(np.float32)


def _register_exp4():
    for op in _dops.OPS:
        if op.name == "EXP4_ANT":
            return op
    spec = _Spec(body=_sq(_sq(_Src0 * _C0 + _C1)) * _C2, reference=_exp4_ref)
    shas = {}
    for ver in ("v3", "v4"):
        s = _DveOpSpec(name="EXP4_ANT", opcode=0, uops=_lower(spec, ver=ver),
                       rd1_en=False)
        shas[ver] = s.sha(ver)
    op = _dops.DveOp("EXP4_ANT", spec, subdim=False, uops_sha=shas)
    _dops.OPS.append(op)
    _dops.CUSTOM_DVE_SPECS[op.name] = spec
    _dops._SUB_OPCODE_FOR_NAME[op.name] = (
        max(_dops._SUB_OPCODE_FOR_NAME.values()) + 1)
    return op


EXP4 = _register_exp4()


EXP_ACT_EARLY = float(os.environ.get("EXP_ACT_EARLY", "0.50"))
EXP_EARLY_CHUNKS = int(os.environ.get("EXP_EARLY_CHUNKS", "16"))
EXP_ACT_LATE = float(os.environ.get("EXP_ACT_LATE", "0.55"))
EXP_LATE_CHUNKS = int(os.environ.get("EXP_LATE_CHUNKS", "16"))


def _exp_engine_pattern():
    """One entry per exp chunk (128 total): True -> ACT, False -> DVE.
    Early chunks lean DVE (ACT busy with qkv drains); late chunks move
    toward 50/50 so both engines finish the last unit together."""
    total_act = EXP_ACT_FRAC * 128
    early_act = EXP_ACT_EARLY * EXP_EARLY_CHUNKS
    late_act = EXP_ACT_LATE * EXP_LATE_CHUNKS
    mid = 128 - EXP_EARLY_CHUNKS - EXP_LATE_CHUNKS
    mid_frac = (total_act - early_act - late_act) / mid
    pat = []
    acc = 0.0
    for i in range(128):
        if i < EXP_EARLY_CHUNKS:
            f = EXP_ACT_EARLY
        elif i >= 128 - EXP_LATE_CHUNKS:
            f = EXP_ACT_LATE
        else:
            f = mid_frac
        acc += f
        if acc >= 1.0 - 1e-9:
            acc -= 1.0
            pat.append(True)
        else:
            pat.append(False)
    return pat


def _build_nc():
    nc = bacc.Bacc(
        "TRN2",
        target_bir_lowering=False,
        debug=False,
        enable_asserts=False,
        num_devices=N_CORES,
    )
    x_d = nc.dram_tensor("x", [P, 4, T], FP8, kind="ExternalInput").ap()
    wqk_d = nc.dram_tensor("wqk", [P, 4, 4, P], FP8, kind="ExternalInput").ap()
    wv_d = nc.dram_tensor("wv", [P, 4, 2 * P], FP8, kind="ExternalInput").ap()
    wp_d = nc.dram_tensor("wp", [P, 2, 4, P], FP8, kind="ExternalInput").ap()
    cqk_d = nc.dram_tensor("cqk", [P, 4], F32, kind="ExternalInput").ap()
    out_d = nc.dram_tensor("out", [P, 4, T], BF16, kind="ExternalOutput").ap()

    pat = _exp_engine_pattern()

    with tile.TileContext(nc) as tc:
        with (
            tc.tile_pool(name="consts", bufs=1) as consts,
            tc.tile_pool(name="xp", bufs=1) as xp,
            tc.tile_pool(name="qkp", bufs=1) as qkp,
            tc.tile_pool(name="vtp", bufs=1) as vtp,
            tc.tile_pool(name="ap", bufs=1) as apool,
            tc.tile_pool(name="wpool", bufs=WPOOL) as wpool,
            tc.tile_pool(name="rhop", bufs=3) as rhop,
            tc.tile_pool(name="repp", bufs=3) as repp,
            tc.tile_pool(name="outp", bufs=OUTP) as outp,
            tc.tile_pool(name="ps_sps", bufs=SPS_BUFS, space="PSUM") as ps_sps,
            tc.tile_pool(name="ps_pj", bufs=1, space="PSUM") as ps_pj,
            tc.tile_pool(name="ps_sh", bufs=1, space="PSUM") as ps_sh,
        ):
            nc.gpsimd.load_library(library_config.attn)

            # ---- DMA in ----
            # DMA_GP=1: input DMAs issue from the GPSIMD queue (Pool DMA
            # config is 25ns/instr vs SP's 565) and x splits into quarters
            # so the 16 parallel DMA engines land it sooner.
            dma_eng = nc.gpsimd if int(os.environ.get("DMA_GP", "0")) \
                else nc.sync
            # DMA transfers are SERIAL on the DMA device: cqk (56ns) must
            # not queue behind the 1.5us x second half -- the first qk
            # drains wait on it.
            x0_eng = nc.gpsimd if int(os.environ.get("X0_GP", "0")) \
                else dma_eng
            x_sb = xp.tile([P, 4, T], FP8)
            x0_eng.dma_start(x_sb[:, :, 0:TH], x_d[:, :, 0:TH])
            wqk = consts.tile([P, 4, 4, P], FP8)
            dma_eng.dma_start(wqk, wqk_d)
            cqk = consts.tile([P, 4], F32)
            dma_eng.dma_start(cqk, cqk_d)
            dma_eng.dma_start(x_sb[:, :, TH:T], x_d[:, :, TH:T])
            wv = consts.tile([P, 4, 2 * P], FP8)
            dma_eng.dma_start(wv, wv_d)
            wp = consts.tile([P, 2, 4, P], FP8)
            dma_eng.dma_start(wp, wp_d)

            # PE p-state warmup while input DMAs land: dummy matmuls on a
            # const tile keep the PE continuously busy so real matmuls start
            # at full clock.
            warm = consts.tile([P, P], FP8)
            nc.vector.memset(warm, 0.0)
            warm2 = consts.tile([P, 512], FP8)
            nc.vector.memset(warm2, 0.0)
            warm_ps = ps_sps.tile([P, 512], F32, tag="sps", name="warm")
            for _ in range(WARMUP):
                nc.tensor.matmul(warm_ps[:, 0:128], lhsT=warm,
                                 rhs=warm2[:, 0:128], start=True, stop=True)

            # ---- qk matmuls + drains ----
            # qk_sb: [128, 2 (kt), 4 (mc), T] fp8.  kt=0 holds q/k data
            # (mc 0: q heads 0/1, 1: q heads 2/3, 2: k heads 0/1, 3: k
            # heads 2/3; head parity is the 64-partition band), kt=1 is
            # ZERO so score matmuls can run DoubleRow with APs
            # [64, 2(kt), m] -- contraction (64 ch + 64 zeros).
            qk_sb = qkp.tile([P, 2, 4, T], FP8)
            # zero the kt=1 planes on the (otherwise idle) GPSIMD engine,
            # in first-use order: k_a, q_a, k_b, q_b
            for mc in (2, 0, 3, 1):
                nc.gpsimd.memset(qk_sb[:, 1, mc, :], 0.0)

            def qk_group(mc, tc2):
                # fused [128, 1024] tile (two tc4 halves) in the sps pool
                qkt = ps_sps.tile([P, 2, 512], F32, tag="sps",
                                  name=f"qk{mc}{tc2}")
                for t2 in range(2):
                    tc4 = tc2 * 2 + t2
                    for kcp in range(2):
                        nc.tensor.matmul(
                            qkt[:, t2, :],
                            lhsT=wqk[:, 2 * kcp : 2 * kcp + 2, mc, :],
                            rhs=x_sb[:, 2 * kcp : 2 * kcp + 2,
                                     tc4 * 512 : (tc4 + 1) * 512],
                            start=(kcp == 0), stop=(kcp == 1),
                            perf_mode=DR,
                        )
                dst = qk_sb[:, 0, mc, tc2 * 1024 : (tc2 + 1) * 1024]
                if QK_DRAIN == "act" or (QK_DRAIN == "alt" and mc in (0, 1)) \
                        or (QK_DRAIN == "alt2" and mc in (2, 3)):
                    nc.scalar.activation(
                        dst,
                        qkt.rearrange("p a b -> p (a b)"),
                        AF.Identity,
                        bias=cqk[:, mc : mc + 1],
                        scale=float(SCALE * SQ / SW),
                    )
                else:
                    nc.vector.tensor_scalar(
                        dst,
                        qkt.rearrange("p a b -> p (a b)"),
                        float(SCALE * SQ / SW),
                        cqk[:, mc : mc + 1],
                        ALU.mult, ALU.add,
                    )

            # ---- vt matmuls + drains ----
            # vt_sb: [128 (s%128), 16 (sc), 4 (hd), 128] fp8; cols 64..127
            # are ONES so av rows 64..127 all come out as the rowsum -- a
            # 64-way replicated rowsum that feeds reciprocal directly (no
            # partition_broadcast needed).
            vt_sb = vtp.tile([P, 16, HL, 2 * CH], FP8)
            nc.gpsimd.memset(vt_sb[:, :, :, CH : 2 * CH], 1.0)

            def vt_group4(g):
                # fused tile: 4 sc chunks (= scp pair 2g, 2g+1)
                vtt = ps_sps.tile([P, 4, 2 * P], F32, tag="sps",
                                  name=f"vt{g}")
                for s4 in range(4):
                    sc = g * 4 + s4
                    for kcp in range(2):
                        nc.tensor.matmul(
                            vtt[:, s4, :],
                            lhsT=x_sb[:, 2 * kcp : 2 * kcp + 2,
                                      sc * P : (sc + 1) * P],
                            rhs=wv[:, 2 * kcp : 2 * kcp + 2, :],
                            start=(kcp == 0), stop=(kcp == 1),
                            perf_mode=DR,
                        )
                if VT_DRAIN == "act" or (VT_DRAIN == "alt" and g % 2 == 0):
                    nc.scalar.activation(
                        vt_sb[:, 4 * g : 4 * g + 4, :, 0:CH],
                        vtt.rearrange("p a (h c) -> p a h c", h=HL),
                        AF.Identity,
                    )
                else:
                    nc.vector.tensor_copy(
                        vt_sb[:, 4 * g : 4 * g + 4, :, 0:CH],
                        vtt.rearrange("p a (h c) -> p a h c", h=HL),
                    )

            # qk for the FIRST chunks only (k_a/q_a tc2=0); the remaining
            # 6 qk groups and the 4 vt groups are injected INTO the
            # attention stream (QK_STREAM=1) so the first score matmul
            # fires as soon as the first x half + wqk land, instead of
            # after all 32 qk matmuls.
            QK_STREAM = int(os.environ.get("QK_STREAM", "0"))
            qk_group(2, 0)                 # k_a s 0..1024
            qk_group(0, 0)                 # q_a t 0..1024
            if QK_STREAM:
                stream_extras = {
                    0: lambda: qk_group(2, 1),   # k_a s 1024.. (need g4)
                    1: lambda: vt_group4(0),     # sc 0-3      (need g5)
                    2: lambda: qk_group(3, 0),   # k_b         (need g16)
                    3: lambda: qk_group(1, 0),   # q_b         (need g16)
                    4: lambda: vt_group4(1),     # sc 4-7      (need g7)
                    5: lambda: qk_group(0, 1),   # q_a th1     (need g32)
                    6: lambda: vt_group4(2),     # sc 8-11     (need g9)
                    7: lambda: vt_group4(3),     # sc 12-15    (need g11)
                    8: lambda: qk_group(3, 1),   # k_b s 1024..(need g20)
                    9: lambda: qk_group(1, 1),   # q_b th1     (need g48)
                }
            else:
                qk_group(2, 1)
                qk_group(0, 1)
                for tc2 in range(2):
                    qk_group(3, tc2)
                    qk_group(1, tc2)
                stream_extras = {}
                if VT_POS == "pre":
                    for g in range(4):
                        vt_group4(g)

            # ---- attention ----
            a_sb = apool.tile([P, 2, T], FP8)

            # attention as a software-pipelined chunk stream: av matmuls
            # lag the scores/exp stream by AV_LAG chunk-pairs so PE never
            # waits on the previous unit's last exp at unit boundaries.
            AV_LAG = int(os.environ.get("AV_LAG", "5"))
            units = [(hd, th) for th in range(2) for hd in range(HL)]
            state = {}   # u -> dict(avs, w_ts)
            exp_ctr = [0]

            def unit_geom(u):
                hd, th = units[u]
                b0 = 64 * (hd % 2)
                q_mc = 0 if hd < 2 else 1
                k_mc = 2 if hd < 2 else 3
                return hd, th, b0, q_mc, k_mc

            def emit_chunk(u, scp):
                hd, th, b0, q_mc, k_mc = unit_geom(u)
                toff = th * TH
                if scp == 0:
                    state[u] = dict(
                        avs=ps_sh.tile([P, 2, 512], F32, tag="sh",
                                       name=f"av{hd}{th}"),
                        w_ts={})
                w_t = wpool.tile([P, 2, TH], FP8, name="wt")
                state[u]["w_ts"][scp] = w_t
                split = u >= len(units) - SPLIT_LAST
                for j in range(2):
                    sc = scp * 2 + j
                    sps = ps_sps.tile([P, TH], F32, tag="sps", name="sps")
                    for tq in range(2):
                        nc.tensor.matmul(
                            sps[:, tq * 512 : (tq + 1) * 512],
                            lhsT=qk_sb[b0 : b0 + CH, :, k_mc,
                                       sc * P : (sc + 1) * P],
                            rhs=qk_sb[b0 : b0 + CH, :, q_mc,
                                      toff + tq * 512 : toff + (tq + 1) * 512],
                            start=True, stop=True,
                            perf_mode=DR,
                        )
                    if split:
                        # tail units: halve each chunk across BOTH engines so
                        # the slot frees sooner and av-tq halves unblock early
                        nc.scalar.activation(
                            w_t[:, j, 0:512], sps[:, 0:512], AF.Exp,
                            scale=float(GAMMA))
                        nc.vector._custom_dve(
                            EXP4, out=w_t[:, j, 512:TH], in0=sps[:, 512:TH],
                            s0=float(QC0 * GAMMA), s1=float(QC1),
                            imm2=float(QC2))
                    elif pat[exp_ctr[0]]:
                        nc.scalar.activation(
                            w_t[:, j, :], sps, AF.Exp, scale=float(GAMMA))
                    else:
                        nc.vector._custom_dve(
                            EXP4, out=w_t[:, j, :], in0=sps,
                            s0=float(QC0 * GAMMA), s1=float(QC1),
                            imm2=float(QC2))
                    exp_ctr[0] += 1

            def emit_av(u, scp):
                hd, th, b0, q_mc, k_mc = unit_geom(u)
                avs = state[u]["avs"]
                w_t = state[u]["w_ts"].pop(scp)
                for tq in range(2):
                    nc.tensor.matmul(
                        avs[:, tq, :],
                        lhsT=vt_sb[:, 2 * scp : 2 * scp + 2, hd, :],
                        rhs=w_t[:, :, tq * 512 : (tq + 1) * 512],
                        start=(scp == 0), stop=(scp == 7),
                        perf_mode=DR,
                    )

            def emit_normalize(u, between=None):
                hd, th, b0, q_mc, k_mc = unit_geom(u)
                toff = th * TH
                avs = state[u]["avs"]
                if between is None:
                    # fused across both tq halves: reciprocal of the 64
                    # replicated rowsum rows IS the broadcast recip; then
                    # one multiply [64,2,512]
                    rep = repp.tile([CH, 2, 512], F32, name="rep")
                    nc.vector.reciprocal(rep, avs[CH : 2 * CH, :, :])
                    nc.vector.tensor_tensor(
                        a_sb[CH * (hd % 2) : CH * (hd % 2) + CH, hd // 2,
                             toff : toff + TH],
                        avs[0:CH, :, :], rep, ALU.mult,
                    )
                else:
                    # last unit: split per tq so proj_tc(2) can interleave
                    for tq in range(2):
                        rep = repp.tile([CH, 512], F32, name="rep")
                        nc.vector.reciprocal(rep, avs[CH : 2 * CH, tq, :])
                        nc.vector.tensor_tensor(
                            a_sb[CH * (hd % 2) : CH * (hd % 2) + CH, hd // 2,
                                 toff + tq * 512 : toff + (tq + 1) * 512],
                            avs[0:CH, tq, :], rep, ALU.mult,
                        )
                        if tq == 0:
                            between()
                del state[u]

            def run_attention(extra=()):
                stream = [(u, scp) for u in range(len(units))
                          for scp in range(8)]
                norm_q = []   # units whose avs are done, normalize deferred

                def pop_norm():
                    lu = norm_q.pop(0)
                    emit_normalize(lu)
                    if lu == 3:           # last th0 unit done
                        proj_tc(0)
                    elif lu == 5:
                        proj_tc(1)

                for g, (u, scp) in enumerate(stream):
                    emit_chunk(u, scp)
                    if QK_STREAM:
                        if g in stream_extras:
                            stream_extras[g]()
                    elif VT_POS == "stream" and g < 4:
                        vt_group4(g)
                    elif VT_POS == "spread" and g in VT_SLOTS:
                        vt_group4(VT_SLOTS.index(g))
                    lag = g - AV_LAG
                    if lag >= 0:
                        lu, lscp = stream[lag]
                        emit_av(lu, lscp)
                        if lscp == 7:
                            norm_q.append(lu)
                    if norm_q:
                        lu = norm_q[0]
                        close_g = (lu * 8 + 7) + AV_LAG  # g when avs closed
                        if g >= close_g + NORM_LAG:
                            pop_norm()
                for lu, lscp in stream[-AV_LAG:]:
                    emit_av(lu, lscp)
                    if lscp == 7:
                        norm_q.append(lu)
                while len(norm_q) > 1:
                    pop_norm()
                emit_normalize(norm_q.pop(0),
                               between=lambda: proj_tc(2, tail=True))

            def proj_tc(tc4, tail=False):
                # oc-PAIR tiles with one fused ap-1024 drain each.  Tail
                # blocks (tc4 2,3) borrow the sps ring -- the score stream
                # is over, so its slots are free; early blocks follow
                # PJ_POOL (default: sh ring, whose slot just freed).
                for op2 in range(2):
                    if tail or PJ_POOL == 0:
                        pjt = ps_sps.tile([P, 2, 512], F32, tag="sps",
                                          name=f"pjs{tc4}{op2}")
                    elif PJ_POOL == 1:
                        pjt = ps_pj.tile([P, 2, 512], F32, tag="pj",
                                         name=f"pjs{tc4}{op2}")
                    else:
                        pjt = ps_sh.tile([P, 2, 512], F32, tag="sh",
                                         name=f"pjs{tc4}{op2}")
                    for o2 in range(2):
                        oc = op2 * 2 + o2
                        nc.tensor.matmul(
                            pjt[:, o2, :],
                            lhsT=wp[:, :, oc, :],
                            rhs=a_sb[:, :, tc4 * 512 : (tc4 + 1) * 512],
                            start=True, stop=True,
                            perf_mode=DR,
                        )
                    ot = outp.tile([P, 2, 512], BF16, name="otp")
                    if tc4 == 3 and tail and PJT_SPLIT_LAST:
                        # final block: halve each drain across both engines
                        # and ship each half as its own DMA so the last
                        # transfer starts as early as possible
                        for o2, eng in ((0, "act"), (1, "dve")):
                            if eng == "act":
                                nc.scalar.activation(
                                    ot[:, o2, :], pjt[:, o2, :], AF.Identity,
                                    scale=float(1.0 / (SV * SW)))
                            else:
                                nc.vector.tensor_scalar(
                                    ot[:, o2, :], pjt[:, o2, :],
                                    float(1.0 / (SV * SW)), None, ALU.mult)
                            nc.sync.dma_start(
                                out_d[:, 2 * op2 + o2,
                                      tc4 * 512 : (tc4 + 1) * 512],
                                ot[:, o2, :])
                        continue
                    if tc4 == 3 and tail and PJT3_DVE:
                        # final block: DVE frees first (after the norm
                        # chain); ACT is still on proj2's drains
                        on_act = op2 == 1
                    else:
                        on_act = (tail and PJT_TAIL_ACT) \
                            or PJ_DRAIN == "act" \
                            or (PJ_DRAIN == "alt" and op2 == 0)
                    if on_act:
                        nc.scalar.activation(
                            ot, pjt, AF.Identity,
                            scale=float(1.0 / (SV * SW)))
                    else:
                        nc.vector.tensor_scalar(
                            ot, pjt, float(1.0 / (SV * SW)),
                            None, ALU.mult)
                    nc.sync.dma_start(
                        out_d[:, 2 * op2 : 2 * op2 + 2,
                              tc4 * 512 : (tc4 + 1) * 512], ot)

            run_attention()
            proj_tc(3, tail=True)
    nc.compile()
    return nc


_NC = None
_LAST_RESULTS = None


def _get_nc():
    global _NC
    if _NC is None:
        _NC = _build_nc()
    return _NC


def _fp8(a):
    return np.ascontiguousarray(a.astype(np.float32).astype(E4))


def kernel(x, mask, gn_gamma, gn_beta, qkv_w, qkv_b, proj_w, proj_b,
           _trace=False):
    del mask  # all-True per problem spec
    x = np.asarray(x, np.float32)
    gn_gamma = np.asarray(gn_gamma, np.float32)
    gn_beta = np.asarray(gn_beta, np.float32)
    qkv_w = np.asarray(qkv_w, np.float32)
    qkv_b = np.asarray(qkv_b, np.float32)
    proj_w = np.asarray(proj_w, np.float32)
    proj_b = np.asarray(proj_b, np.float32)

    # exact GroupNorm stats per batch (host, f32)
    xg = x.reshape(B, G, C // G, T)
    mu = xg.mean(axis=(2, 3))                      # [B, G]
    var = xg.var(axis=(2, 3))                      # [B, G]
    s_bg = 1.0 / np.sqrt(var + EPS)                # [B, G]
    s_bc = np.repeat(s_bg, C // G, axis=1) * gn_gamma[None, :]      # [B, C]
    off_bc = gn_beta[None, :] - np.repeat(mu * s_bg, C // G, axis=1) \
        * gn_gamma[None, :]                        # [B, C]

    in_maps = []
    v_bias_term = {}
    for core in range(N_CORES):
        b, hh = core // 2, core % 2
        heads = [hh * HL + i for i in range(HL)]
        # column order for q/k: [head][ch]; mc blocks = head pairs
        q_rows = np.concatenate(
            [np.arange(h * 192, h * 192 + 64) for h in heads])
        k_rows = q_rows + 64
        v_rows = np.concatenate([np.arange(h * 192 + 128, h * 192 + 192)
                                 for h in heads])

        s = s_bc[b]                                # [C]
        off = off_bc[b]                            # [C]

        wq = qkv_w[q_rows] * s[None, :]            # [256, 512]
        wk = qkv_w[k_rows] * s[None, :]
        wv_ = qkv_w[v_rows] * s[None, :]
        # wqk dram layout [p(c%128), kc(c//128), mc, m(128)]
        wqk_m = np.concatenate([wq, wk], 0)        # [512(m), 512(c)]
        wqk_t = (wqk_m.T.reshape(4, P, 4, P)
                 .transpose(1, 0, 2, 3))           # [p, kc, mc, m]
        wqk_t = wqk_t * SW
        wv_t = wv_.T.reshape(4, P, 2 * P).transpose(1, 0, 2) * SW
        # proj columns for this half, reordered to head-band x ch
        wp_cols = proj_w[:, [hh * 256 + i for i in range(256)]]  # [512, 256]
        # a_sb rows: [hd%2 band (64), hd//2 ktile]: channel (hd, ch) sits at
        # row 64*(hd%2)+ch of ktile hd//2 -> input index hd*64+ch
        perm = np.array([(kt * 2 + band) * 64 + ch
                         for kt in range(2) for band in range(2)
                         for ch in range(64)])
        # rows of wp lhsT tile [p, kt, oc, m]: p = 64*band+ch
        wp_in = wp_cols[:, perm]                   # [512 out, 256 perm-in]
        wp_t = (wp_in.T.reshape(2, P, 4, P)
                .transpose(1, 0, 2, 3)) * SW       # [p, kt, oc, m]

        cq = (qkv_w[q_rows] @ off + qkv_b[q_rows]) * SCALE * SQ
        ck = (qkv_w[k_rows] @ off + qkv_b[k_rows]) * SCALE * SQ
        cqk = np.stack([cq[:P], cq[P:], ck[:P], ck[P:]], axis=1)  # [128, 4]

        x_t = x[b].reshape(4, P, T).transpose(1, 0, 2)

        in_maps.append(dict(
            x=_fp8(x_t),
            wqk=_fp8(wqk_t),
            wv=_fp8(wv_t),
            wp=_fp8(wp_t),
            cqk=np.ascontiguousarray(cqk, dtype=np.float32),
        ))
        # v bias + GN-offset contribution through v, exact on host:
        cv = qkv_w[v_rows] @ off + qkv_b[v_rows]   # [256]
        v_bias_term[core] = proj_w[:, hh * 256 : hh * 256 + 256] @ cv  # [512]

    nc = _get_nc()
    res = run_bass_kernel_spmd(nc, in_maps, core_ids=list(range(N_CORES)),
                               trace=_trace)
    global _LAST_RESULTS
    _LAST_RESULTS = res
    out = np.empty((B, C, T), np.float32)
    for b in range(B):
        r0 = res.results[2 * b]["out"].astype(np.float32)
        r1 = res.results[2 * b + 1]["out"].astype(np.float32)
        const = (v_bias_term[2 * b] + v_bias_term[2 * b + 1]
                 + proj_b)[:, None]
        out[b] = (x[b]
                  + r0.transpose(1, 0, 2).reshape(C, T)
                  + r1.transpose(1, 0, 2).reshape(C, T)
                  + const)
    return out


# revision 44
# speedup vs baseline: 1.0720x; 1.0060x over previous
"""AttentionBlock (GroupNorm -> qkv -> softmax attention -> proj + residual)
for Trainium2, 8 NeuronCores, fp8 DoubleRow edition.

Sharding: core = (batch b, head-half hh): each core handles 1 of 4 batches
and 4 of 8 heads, computing a partial projection output; the host sums the
two partials per batch and adds the residual x and proj_b.

Device-side structure (per core):
 - GroupNorm is folded into the weights on the HOST: h = s*x + off with
   per-(batch,channel) s/off from exact f32 stats, so W' = W*diag(s) (fp8)
   and per-out-channel biases ride the PSUM->SBUF drains.  x ships as fp8.
 - ALL matmuls (qkv/scores/av/proj) run in fp8e4 DoubleRow perf mode
   (0.5 cycles/row).  The score matmuls have only a 64-deep contraction
   (per-head channels); DoubleRow needs a k-tile PAIR, so q/k live in
   [128, 2(kt), 4(mc), T] tiles whose kt=1 plane is zero: lhsT/rhs APs
   [64, 2, m] contract over (64 ch + 64 zeros) -- numerically exact, and
   the cost halves.
 - exp(scores) is split between the ACT engine (native Exp) and the DVE
   (a custom quartic c2*(c0*x+c1)^4 DVE op registered at import time).
 - softmax normalization: rowsums come free via a ones-column in v^T; per
   unit ONE fused reciprocal [1,2,512] (DVE), ONE partition_broadcast
   (GPSIMD) and ONE multiply [64,2,512] (DVE) cover both t-halves.

The mask input is all-True per the problem spec, a numeric no-op.  q/k/GN
biases are folded exactly; v/proj biases are added exactly on the host.
"""

import os
import numpy as np
import ml_dtypes

import concourse.bass as bass
import concourse.tile as tile
from concourse import bacc, mybir, library_config
from concourse.bass_utils import run_bass_kernel_spmd

F32 = mybir.dt.float32
BF16 = mybir.dt.bfloat16
FP8 = mybir.dt.float8e4
AF = mybir.ActivationFunctionType
ALU = mybir.AluOpType
DR = mybir.MatmulPerfMode.DoubleRow
E4 = ml_dtypes.float8_e4m3

B, C, T, H = 4, 512, 2048, 8
CH = 64
G = 32
EPS = 1e-5
HL = 4                 # heads per core
P = 128
TH = T // 2            # 1024, t-half handled per (hd, th)
N_CORES = 8

# fp8 range scales
SW = 64.0              # weight upscale for fp8 (qkv + proj weights)
SQ = 4.0               # q/k sbuf upscale (on top of 1/sqrt(sqrt(ch)))
SV = 64.0              # v sbuf upscale (=SW so vt drain is a plain copy)
GAMMA = 1.0 / (SQ * SQ)  # descale applied inside exp
SCALE = 1.0 / np.sqrt(np.sqrt(CH))

# quartic exp approximation constants (minimax on [-1.7, 1.7])
QC0, QC1, QC2 = 0.24274105, 1.02873227, 1.04374374

# exp engine split: chunk i of 16 per (hd, th) goes to ACT if pattern bit set
EXP_ACT_FRAC = float(os.environ.get("EXP_ACT_FRAC", "0.595"))
# drain engine choices ("act" or "dve")
QK_DRAIN = os.environ.get("QK_DRAIN", "alt")
VT_DRAIN = os.environ.get("VT_DRAIN", "act")
PJ_DRAIN = os.environ.get("PJ_DRAIN", "alt")
# tail proj blocks drain on ACT only, so the last unit's normalize chain
# never queues behind a proj drain on the in-order DVE
PJT_TAIL_ACT = int(os.environ.get("PJT_TAIL_ACT", "1"))

# split the very last proj block's drains into ap-512 halves on both
# engines so the final out-DMA starts sooner
PJT_SPLIT_LAST = int(os.environ.get("PJT_SPLIT_LAST", "0"))
# final proj block: first drain on DVE (frees before ACT at the tail)
PJT3_DVE = int(os.environ.get("PJT3_DVE", "1"))
VT_POS = os.environ.get("VT_POS", "spread")
WPOOL = int(os.environ.get("WPOOL", "12"))
VT_SLOTS = tuple(int(v) for v in os.environ.get("VT_SLOTS", "1,3,5,7").split(","))
WARMUP = int(os.environ.get("WARMUP", "12"))
NORM_LAG = int(os.environ.get("NORM_LAG", "0"))
SPLIT_LAST = int(os.environ.get("SPLIT_LAST", "0"))
OUTP = int(os.environ.get("OUTP", "6"))
# PJ_POOL: where proj psum tiles come from. 0 = borrow the sps ring
# (stalls the score stream ~1us per burst); 2 = borrow the sh (avs) ring
# (proj naturally waits for the freshly-freed avs slot).  1 = dedicated
# pool, only with SPS_BUFS=2 — measured much worse, keep for reference.
PJ_POOL = int(os.environ.get("PJ_POOL", "2"))
SPS_BUFS = int(os.environ.get("SPS_BUFS", "2" if PJ_POOL == 1 else "3"))

# ---- custom DVE op: EXP4_ANT = c2*(c0*x+c1)^4 ------------------------------
from concourse import dve_ops as _dops
from concourse.dve_spec import Spec as _Spec, Src0 as _Src0, C0 as _C0, \
    C1 as _C1, C2 as _C2, sq as _sq, lower as _lower
from concourse.dve_uop import DveOpSpec as _DveOpSpec


def _exp4_ref(in0, in1, c0, c1, c2):
    y = np.square(np.square(in0.astype(np.float32) * c0 + c1)) * c2
    return y.astype(np.float32)


def _register_exp4():
    for op in _dops.OPS:
        if op.name == "EXP4_ANT":
            return op
    spec = _Spec(body=_sq(_sq(_Src0 * _C0 + _C1)) * _C2, reference=_exp4_ref)
    shas = {}
    for ver in ("v3", "v4"):
        s = _DveOpSpec(name="EXP4_ANT", opcode=0, uops=_lower(spec, ver=ver),
                       rd1_en=False)
        shas[ver] = s.sha(ver)
    op = _dops.DveOp("EXP4_ANT", spec, subdim=False, uops_sha=shas)
    _dops.OPS.append(op)
    _dops.CUSTOM_DVE_SPECS[op.name] = spec
    _dops._SUB_OPCODE_FOR_NAME[op.name] = (
        max(_dops._SUB_OPCODE_FOR_NAME.values()) + 1)
    return op


EXP4 = _register_exp4()


EXP_ACT_EARLY = float(os.environ.get("EXP_ACT_EARLY", "0.50"))
EXP_EARLY_CHUNKS = int(os.environ.get("EXP_EARLY_CHUNKS", "16"))
EXP_ACT_LATE = float(os.environ.get("EXP_ACT_LATE", "0.55"))
EXP_LATE_CHUNKS = int(os.environ.get("EXP_LATE_CHUNKS", "16"))


def _exp_engine_pattern():
    """One entry per exp chunk (128 total): True -> ACT, False -> DVE.
    Early chunks lean DVE (ACT busy with qkv drains); late chunks move
    toward 50/50 so both engines finish the last unit together."""
    if os.environ.get("PATTERN", "") == "unit":
        # unit-position-aware: each unit's norm (recip+mult) lands on DVE
        # while the NEXT unit's chunks 4..11 stream, so give ACT a larger
        # share there
        hi = float(os.environ.get("PAT_HI", "0.75"))
        lo = 2 * EXP_ACT_FRAC - hi
        pat = []
        acc = 0.0
        for i in range(128):
            pos = i % 16
            f = hi if 4 <= pos < 12 else lo
            acc += f
            if acc >= 1.0 - 1e-9:
                acc -= 1.0
                pat.append(True)
            else:
                pat.append(False)
        return pat
    total_act = EXP_ACT_FRAC * 128
    early_act = EXP_ACT_EARLY * EXP_EARLY_CHUNKS
    late_act = EXP_ACT_LATE * EXP_LATE_CHUNKS
    mid = 128 - EXP_EARLY_CHUNKS - EXP_LATE_CHUNKS
    mid_frac = (total_act - early_act - late_act) / mid
    pat = []
    acc = 0.0
    for i in range(128):
        if i < EXP_EARLY_CHUNKS:
            f = EXP_ACT_EARLY
        elif i >= 128 - EXP_LATE_CHUNKS:
            f = EXP_ACT_LATE
        else:
            f = mid_frac
        acc += f
        if acc >= 1.0 - 1e-9:
            acc -= 1.0
            pat.append(True)
        else:
            pat.append(False)
    return pat


def _build_nc():
    nc = bacc.Bacc(
        "TRN2",
        target_bir_lowering=False,
        debug=False,
        enable_asserts=False,
        num_devices=N_CORES,
    )
    x_d = nc.dram_tensor("x", [P, 4, T], FP8, kind="ExternalInput").ap()
    wqk_d = nc.dram_tensor("wqk", [P, 4, 4, P], FP8, kind="ExternalInput").ap()
    wv_d = nc.dram_tensor("wv", [P, 4, 2 * P], FP8, kind="ExternalInput").ap()
    wp_d = nc.dram_tensor("wp", [P, 2, 4, P], FP8, kind="ExternalInput").ap()
    cqk_d = nc.dram_tensor("cqk", [P, 4], F32, kind="ExternalInput").ap()
    out_d = nc.dram_tensor("out", [P, 4, T], BF16, kind="ExternalOutput").ap()

    pat = _exp_engine_pattern()

    with tile.TileContext(nc) as tc:
        with (
            tc.tile_pool(name="consts", bufs=1) as consts,
            tc.tile_pool(name="xp", bufs=1) as xp,
            tc.tile_pool(name="qkp", bufs=1) as qkp,
            tc.tile_pool(name="vtp", bufs=1) as vtp,
            tc.tile_pool(name="ap", bufs=1) as apool,
            tc.tile_pool(name="wpool", bufs=WPOOL) as wpool,
            tc.tile_pool(name="rhop", bufs=3) as rhop,
            tc.tile_pool(name="repp", bufs=3) as repp,
            tc.tile_pool(name="outp", bufs=OUTP) as outp,
            tc.tile_pool(name="ps_sps", bufs=SPS_BUFS, space="PSUM") as ps_sps,
            tc.tile_pool(name="ps_pj", bufs=1, space="PSUM") as ps_pj,
            tc.tile_pool(name="ps_sh", bufs=1, space="PSUM") as ps_sh,
        ):
            nc.gpsimd.load_library(library_config.attn)

            # ---- DMA in ----
            # DMA_GP=1: input DMAs issue from the GPSIMD queue (Pool DMA
            # config is 25ns/instr vs SP's 565) and x splits into quarters
            # so the 16 parallel DMA engines land it sooner.
            dma_eng = nc.gpsimd if int(os.environ.get("DMA_GP", "0")) \
                else nc.sync
            # DMA transfers are SERIAL on the DMA device: cqk (56ns) must
            # not queue behind the 1.5us x second half -- the first qk
            # drains wait on it.
            x_sb = xp.tile([P, 4, T], FP8)
            wqk = consts.tile([P, 4, 4, P], FP8)
            cqk = consts.tile([P, 4], F32)
            wv = consts.tile([P, 4, 2 * P], FP8)
            wp = consts.tile([P, 2, 4, P], FP8)
            if int(os.environ.get("DMA_MIX", "0")):
                # spread input DMAs over DIFFERENT engines' DGE queues so
                # the transfers overlap on the 16-engine DMA device instead
                # of serializing on one queue
                nc.sync.dma_start(x_sb[:, :, 0:TH], x_d[:, :, 0:TH])
                nc.gpsimd.dma_start(wqk, wqk_d)
                nc.gpsimd.dma_start(cqk, cqk_d)
                nc.scalar.dma_start(x_sb[:, :, TH:T], x_d[:, :, TH:T])
                nc.gpsimd.dma_start(wv, wv_d)
                nc.gpsimd.dma_start(wp, wp_d)
            else:
                dma_eng.dma_start(x_sb[:, :, 0:TH], x_d[:, :, 0:TH])
                dma_eng.dma_start(wqk, wqk_d)
                dma_eng.dma_start(cqk, cqk_d)
                dma_eng.dma_start(x_sb[:, :, TH:T], x_d[:, :, TH:T])
                dma_eng.dma_start(wv, wv_d)
                dma_eng.dma_start(wp, wp_d)

            # PE p-state warmup while input DMAs land: dummy matmuls on a
            # const tile keep the PE continuously busy so real matmuls start
            # at full clock.
            warm = consts.tile([P, P], FP8)
            nc.vector.memset(warm, 0.0)
            warm2 = consts.tile([P, 512], FP8)
            nc.vector.memset(warm2, 0.0)
            warm_ps = ps_sps.tile([P, 512], F32, tag="sps", name="warm")
            for _ in range(WARMUP):
                nc.tensor.matmul(warm_ps[:, 0:128], lhsT=warm,
                                 rhs=warm2[:, 0:128], start=True, stop=True)

            # ---- qk matmuls + drains ----
            # qk_sb: [128, 2 (kt), 4 (mc), T] fp8.  kt=0 holds q/k data
            # (mc 0: q heads 0/1, 1: q heads 2/3, 2: k heads 0/1, 3: k
            # heads 2/3; head parity is the 64-partition band), kt=1 is
            # ZERO so score matmuls can run DoubleRow with APs
            # [64, 2(kt), m] -- contraction (64 ch + 64 zeros).
            qk_sb = qkp.tile([P, 2, 4, T], FP8)
            # zero the kt=1 planes on the (otherwise idle) GPSIMD engine,
            # in first-use order: k_a, q_a, k_b, q_b
            for mc in (2, 0, 3, 1):
                nc.gpsimd.memset(qk_sb[:, 1, mc, :], 0.0)

            def qk_group(mc, tc2):
                # fused [128, 1024] tile (two tc4 halves) in the sps pool
                qkt = ps_sps.tile([P, 2, 512], F32, tag="sps",
                                  name=f"qk{mc}{tc2}")
                for t2 in range(2):
                    tc4 = tc2 * 2 + t2
                    for kcp in range(2):
                        nc.tensor.matmul(
                            qkt[:, t2, :],
                            lhsT=wqk[:, 2 * kcp : 2 * kcp + 2, mc, :],
                            rhs=x_sb[:, 2 * kcp : 2 * kcp + 2,
                                     tc4 * 512 : (tc4 + 1) * 512],
                            start=(kcp == 0), stop=(kcp == 1),
                            perf_mode=DR,
                        )
                dst = qk_sb[:, 0, mc, tc2 * 1024 : (tc2 + 1) * 1024]
                if QK_DRAIN == "act" or (QK_DRAIN == "alt" and mc in (0, 1)) \
                        or (QK_DRAIN == "alt2" and mc in (2, 3)):
                    nc.scalar.activation(
                        dst,
                        qkt.rearrange("p a b -> p (a b)"),
                        AF.Identity,
                        bias=cqk[:, mc : mc + 1],
                        scale=float(SCALE * SQ / SW),
                    )
                else:
                    nc.vector.tensor_scalar(
                        dst,
                        qkt.rearrange("p a b -> p (a b)"),
                        float(SCALE * SQ / SW),
                        cqk[:, mc : mc + 1],
                        ALU.mult, ALU.add,
                    )

            # ---- vt matmuls + drains ----
            # vt_sb: [128 (s%128), 16 (sc), 4 (hd), 128] fp8; cols 64..127
            # are ONES so av rows 64..127 all come out as the rowsum -- a
            # 64-way replicated rowsum that feeds reciprocal directly (no
            # partition_broadcast needed).
            vt_sb = vtp.tile([P, 16, HL, 2 * CH], FP8)
            nc.gpsimd.memset(vt_sb[:, :, :, CH : 2 * CH], 1.0)

            def vt_group4(g):
                # fused tile: 4 sc chunks (= scp pair 2g, 2g+1)
                vtt = ps_sps.tile([P, 4, 2 * P], F32, tag="sps",
                                  name=f"vt{g}")
                for s4 in range(4):
                    sc = g * 4 + s4
                    for kcp in range(2):
                        nc.tensor.matmul(
                            vtt[:, s4, :],
                            lhsT=x_sb[:, 2 * kcp : 2 * kcp + 2,
                                      sc * P : (sc + 1) * P],
                            rhs=wv[:, 2 * kcp : 2 * kcp + 2, :],
                            start=(kcp == 0), stop=(kcp == 1),
                            perf_mode=DR,
                        )
                if VT_DRAIN == "act" or (VT_DRAIN == "alt" and g % 2 == 0):
                    nc.scalar.activation(
                        vt_sb[:, 4 * g : 4 * g + 4, :, 0:CH],
                        vtt.rearrange("p a (h c) -> p a h c", h=HL),
                        AF.Identity,
                    )
                else:
                    nc.vector.tensor_copy(
                        vt_sb[:, 4 * g : 4 * g + 4, :, 0:CH],
                        vtt.rearrange("p a (h c) -> p a h c", h=HL),
                    )

            # qk for the FIRST chunks only (k_a/q_a tc2=0); the remaining
            # 6 qk groups and the 4 vt groups are injected INTO the
            # attention stream (QK_STREAM=1) so the first score matmul
            # fires as soon as the first x half + wqk land, instead of
            # after all 32 qk matmuls.
            QK_STREAM = int(os.environ.get("QK_STREAM", "0"))
            qk_group(2, 0)                 # k_a s 0..1024
            qk_group(0, 0)                 # q_a t 0..1024
            if QK_STREAM:
                stream_extras = {
                    0: lambda: qk_group(2, 1),   # k_a s 1024.. (need g4)
                    1: lambda: vt_group4(0),     # sc 0-3      (need g5)
                    2: lambda: qk_group(3, 0),   # k_b         (need g16)
                    3: lambda: qk_group(1, 0),   # q_b         (need g16)
                    4: lambda: vt_group4(1),     # sc 4-7      (need g7)
                    5: lambda: qk_group(0, 1),   # q_a th1     (need g32)
                    6: lambda: vt_group4(2),     # sc 8-11     (need g9)
                    7: lambda: vt_group4(3),     # sc 12-15    (need g11)
                    8: lambda: qk_group(3, 1),   # k_b s 1024..(need g20)
                    9: lambda: qk_group(1, 1),   # q_b th1     (need g48)
                }
            else:
                qk_group(2, 1)
                qk_group(0, 1)
                for tc2 in range(2):
                    qk_group(3, tc2)
                    qk_group(1, tc2)
                stream_extras = {}
                if VT_POS == "pre":
                    for g in range(4):
                        vt_group4(g)

            # ---- attention ----
            a_sb = apool.tile([P, 2, T], FP8)

            # attention as a software-pipelined chunk stream: av matmuls
            # lag the scores/exp stream by AV_LAG chunk-pairs so PE never
            # waits on the previous unit's last exp at unit boundaries.
            AV_LAG = int(os.environ.get("AV_LAG", "5"))
            units = [(hd, th) for th in range(2) for hd in range(HL)]
            state = {}   # u -> dict(avs, w_ts)
            exp_ctr = [0]

            def unit_geom(u):
                hd, th = units[u]
                b0 = 64 * (hd % 2)
                q_mc = 0 if hd < 2 else 1
                k_mc = 2 if hd < 2 else 3
                return hd, th, b0, q_mc, k_mc

            def emit_chunk(u, scp):
                hd, th, b0, q_mc, k_mc = unit_geom(u)
                toff = th * TH
                if scp == 0:
                    state[u] = dict(
                        avs=ps_sh.tile([P, 2, 512], F32, tag="sh",
                                       name=f"av{hd}{th}"),
                        w_ts={})
                w_t = wpool.tile([P, 2, TH], FP8, name="wt")
                state[u]["w_ts"][scp] = w_t
                split = u >= len(units) - SPLIT_LAST
                for j in range(2):
                    sc = scp * 2 + j
                    sps = ps_sps.tile([P, TH], F32, tag="sps", name="sps")
                    for tq in range(2):
                        nc.tensor.matmul(
                            sps[:, tq * 512 : (tq + 1) * 512],
                            lhsT=qk_sb[b0 : b0 + CH, :, k_mc,
                                       sc * P : (sc + 1) * P],
                            rhs=qk_sb[b0 : b0 + CH, :, q_mc,
                                      toff + tq * 512 : toff + (tq + 1) * 512],
                            start=True, stop=True,
                            perf_mode=DR,
                        )
                    if split:
                        # tail units: halve each chunk across BOTH engines so
                        # the slot frees sooner and av-tq halves unblock early
                        nc.scalar.activation(
                            w_t[:, j, 0:512], sps[:, 0:512], AF.Exp,
                            scale=float(GAMMA))
                        nc.vector._custom_dve(
                            EXP4, out=w_t[:, j, 512:TH], in0=sps[:, 512:TH],
                            s0=float(QC0 * GAMMA), s1=float(QC1),
                            imm2=float(QC2))
                    elif pat[exp_ctr[0]]:
                        nc.scalar.activation(
                            w_t[:, j, :], sps, AF.Exp, scale=float(GAMMA))
                    else:
                        nc.vector._custom_dve(
                            EXP4, out=w_t[:, j, :], in0=sps,
                            s0=float(QC0 * GAMMA), s1=float(QC1),
                            imm2=float(QC2))
                    exp_ctr[0] += 1

            def emit_av(u, scp):
                hd, th, b0, q_mc, k_mc = unit_geom(u)
                avs = state[u]["avs"]
                w_t = state[u]["w_ts"].pop(scp)
                for tq in range(2):
                    nc.tensor.matmul(
                        avs[:, tq, :],
                        lhsT=vt_sb[:, 2 * scp : 2 * scp + 2, hd, :],
                        rhs=w_t[:, :, tq * 512 : (tq + 1) * 512],
                        start=(scp == 0), stop=(scp == 7),
                        perf_mode=DR,
                    )

            def emit_normalize(u, between=None):
                hd, th, b0, q_mc, k_mc = unit_geom(u)
                toff = th * TH
                avs = state[u]["avs"]
                if between is None:
                    # fused across both tq halves: reciprocal of the 64
                    # replicated rowsum rows IS the broadcast recip; then
                    # one multiply [64,2,512]
                    rep = repp.tile([CH, 2, 512], F32, name="rep")
                    nc.vector.reciprocal(rep, avs[CH : 2 * CH, :, :])
                    nc.vector.tensor_tensor(
                        a_sb[CH * (hd % 2) : CH * (hd % 2) + CH, hd // 2,
                             toff : toff + TH],
                        avs[0:CH, :, :], rep, ALU.mult,
                    )
                else:
                    # last unit: split per tq so proj_tc(2) can interleave
                    for tq in range(2):
                        rep = repp.tile([CH, 512], F32, name="rep")
                        nc.vector.reciprocal(rep, avs[CH : 2 * CH, tq, :])
                        nc.vector.tensor_tensor(
                            a_sb[CH * (hd % 2) : CH * (hd % 2) + CH, hd // 2,
                                 toff + tq * 512 : toff + (tq + 1) * 512],
                            avs[0:CH, tq, :], rep, ALU.mult,
                        )
                        if tq == 0:
                            between()
                del state[u]

            def run_attention(extra=()):
                stream = [(u, scp) for u in range(len(units))
                          for scp in range(8)]
                norm_q = []   # units whose avs are done, normalize deferred

                def pop_norm():
                    lu = norm_q.pop(0)
                    emit_normalize(lu)
                    if lu == 3:           # last th0 unit done
                        proj_tc(0)
                    elif lu == 5:
                        proj_tc(1)

                for g, (u, scp) in enumerate(stream):
                    emit_chunk(u, scp)
                    if QK_STREAM:
                        if g in stream_extras:
                            stream_extras[g]()
                    elif VT_POS == "stream" and g < 4:
                        vt_group4(g)
                    elif VT_POS == "spread" and g in VT_SLOTS:
                        vt_group4(VT_SLOTS.index(g))
                    lag = g - AV_LAG
                    if lag >= 0:
                        lu, lscp = stream[lag]
                        emit_av(lu, lscp)
                        if lscp == 7:
                            norm_q.append(lu)
                    if norm_q:
                        lu = norm_q[0]
                        close_g = (lu * 8 + 7) + AV_LAG  # g when avs closed
                        if g >= close_g + NORM_LAG:
                            pop_norm()
                for lu, lscp in stream[-AV_LAG:]:
                    emit_av(lu, lscp)
                    if lscp == 7:
                        norm_q.append(lu)
                while len(norm_q) > 1:
                    pop_norm()
                emit_normalize(norm_q.pop(0),
                               between=lambda: proj_tc(2, tail=True))

            def proj_tc(tc4, tail=False):
                # oc-PAIR tiles with one fused ap-1024 drain each.  Tail
                # blocks (tc4 2,3) borrow the sps ring -- the score stream
                # is over, so its slots are free; early blocks follow
                # PJ_POOL (default: sh ring, whose slot just freed).
                for op2 in range(2):
                    if tail or PJ_POOL == 0:
                        pjt = ps_sps.tile([P, 2, 512], F32, tag="sps",
                                          name=f"pjs{tc4}{op2}")
                    elif PJ_POOL == 1:
                        pjt = ps_pj.tile([P, 2, 512], F32, tag="pj",
                                         name=f"pjs{tc4}{op2}")
                    else:
                        pjt = ps_sh.tile([P, 2, 512], F32, tag="sh",
                                         name=f"pjs{tc4}{op2}")
                    for o2 in range(2):
                        oc = op2 * 2 + o2
                        nc.tensor.matmul(
                            pjt[:, o2, :],
                            lhsT=wp[:, :, oc, :],
                            rhs=a_sb[:, :, tc4 * 512 : (tc4 + 1) * 512],
                            start=True, stop=True,
                            perf_mode=DR,
                        )
                    ot = outp.tile([P, 2, 512], BF16, name="otp")
                    if tc4 == 3 and tail and PJT_SPLIT_LAST:
                        # final block: halve each drain across both engines
                        # and ship each half as its own DMA so the last
                        # transfer starts as early as possible
                        for o2, eng in ((0, "act"), (1, "dve")):
                            if eng == "act":
                                nc.scalar.activation(
                                    ot[:, o2, :], pjt[:, o2, :], AF.Identity,
                                    scale=float(1.0 / (SV * SW)))
                            else:
                                nc.vector.tensor_scalar(
                                    ot[:, o2, :], pjt[:, o2, :],
                                    float(1.0 / (SV * SW)), None, ALU.mult)
                            nc.sync.dma_start(
                                out_d[:, 2 * op2 + o2,
                                      tc4 * 512 : (tc4 + 1) * 512],
                                ot[:, o2, :])
                        continue
                    if tc4 == 3 and tail and PJT3_DVE:
                        # final block: DVE frees first (after the norm
                        # chain); ACT is still on proj2's drains
                        on_act = op2 == 1
                    else:
                        on_act = (tail and PJT_TAIL_ACT) \
                            or PJ_DRAIN == "act" \
                            or (PJ_DRAIN == "alt" and op2 == 0)
                    if on_act:
                        nc.scalar.activation(
                            ot, pjt, AF.Identity,
                            scale=float(1.0 / (SV * SW)))
                    else:
                        nc.vector.tensor_scalar(
                            ot, pjt, float(1.0 / (SV * SW)),
                            None, ALU.mult)
                    nc.sync.dma_start(
                        out_d[:, 2 * op2 : 2 * op2 + 2,
                              tc4 * 512 : (tc4 + 1) * 512], ot)

            run_attention()
            proj_tc(3, tail=True)
    nc.compile()
    return nc


_NC = None
_LAST_RESULTS = None


def _get_nc():
    global _NC
    if _NC is None:
        _NC = _build_nc()
    return _NC


def _fp8(a):
    return np.ascontiguousarray(a.astype(np.float32).astype(E4))


def kernel(x, mask, gn_gamma, gn_beta, qkv_w, qkv_b, proj_w, proj_b,
           _trace=False):
    del mask  # all-True per problem spec
    x = np.asarray(x, np.float32)
    gn_gamma = np.asarray(gn_gamma, np.float32)
    gn_beta = np.asarray(gn_beta, np.float32)
    qkv_w = np.asarray(qkv_w, np.float32)
    qkv_b = np.asarray(qkv_b, np.float32)
    proj_w = np.asarray(proj_w, np.float32)
    proj_b = np.asarray(proj_b, np.float32)

    # exact GroupNorm stats per batch (host, f32)
    xg = x.reshape(B, G, C // G, T)
    mu = xg.mean(axis=(2, 3))                      # [B, G]
    var = xg.var(axis=(2, 3))                      # [B, G]
    s_bg = 1.0 / np.sqrt(var + EPS)                # [B, G]
    s_bc = np.repeat(s_bg, C // G, axis=1) * gn_gamma[None, :]      # [B, C]
    off_bc = gn_beta[None, :] - np.repeat(mu * s_bg, C // G, axis=1) \
        * gn_gamma[None, :]                        # [B, C]

    in_maps = []
    v_bias_term = {}
    for core in range(N_CORES):
        b, hh = core // 2, core % 2
        heads = [hh * HL + i for i in range(HL)]
        # column order for q/k: [head][ch]; mc blocks = head pairs
        q_rows = np.concatenate(
            [np.arange(h * 192, h * 192 + 64) for h in heads])
        k_rows = q_rows + 64
        v_rows = np.concatenate([np.arange(h * 192 + 128, h * 192 + 192)
                                 for h in heads])

        s = s_bc[b]                                # [C]
        off = off_bc[b]                            # [C]

        wq = qkv_w[q_rows] * s[None, :]            # [256, 512]
        wk = qkv_w[k_rows] * s[None, :]
        wv_ = qkv_w[v_rows] * s[None, :]
        # wqk dram layout [p(c%128), kc(c//128), mc, m(128)]
        wqk_m = np.concatenate([wq, wk], 0)        # [512(m), 512(c)]
        wqk_t = (wqk_m.T.reshape(4, P, 4, P)
                 .transpose(1, 0, 2, 3))           # [p, kc, mc, m]
        wqk_t = wqk_t * SW
        wv_t = wv_.T.reshape(4, P, 2 * P).transpose(1, 0, 2) * SW
        # proj columns for this half, reordered to head-band x ch
        wp_cols = proj_w[:, [hh * 256 + i for i in range(256)]]  # [512, 256]
        # a_sb rows: [hd%2 band (64), hd//2 ktile]: channel (hd, ch) sits at
        # row 64*(hd%2)+ch of ktile hd//2 -> input index hd*64+ch
        perm = np.array([(kt * 2 + band) * 64 + ch
                         for kt in range(2) for band in range(2)
                         for ch in range(64)])
        # rows of wp lhsT tile [p, kt, oc, m]: p = 64*band+ch
        wp_in = wp_cols[:, perm]                   # [512 out, 256 perm-in]
        wp_t = (wp_in.T.reshape(2, P, 4, P)
                .transpose(1, 0, 2, 3)) * SW       # [p, kt, oc, m]

        cq = (qkv_w[q_rows] @ off + qkv_b[q_rows]) * SCALE * SQ
        ck = (qkv_w[k_rows] @ off + qkv_b[k_rows]) * SCALE * SQ
        cqk = np.stack([cq[:P], cq[P:], ck[:P], ck[P:]], axis=1)  # [128, 4]

        x_t = x[b].reshape(4, P, T).transpose(1, 0, 2)

        in_maps.append(dict(
            x=_fp8(x_t),
            wqk=_fp8(wqk_t),
            wv=_fp8(wv_t),
            wp=_fp8(wp_t),
            cqk=np.ascontiguousarray(cqk, dtype=np.float32),
        ))
        # v bias + GN-offset contribution through v, exact on host:
        cv = qkv_w[v_rows] @ off + qkv_b[v_rows]   # [256]
        v_bias_term[core] = proj_w[:, hh * 256 : hh * 256 + 256] @ cv  # [512]

    nc = _get_nc()
    res = run_bass_kernel_spmd(nc, in_maps, core_ids=list(range(N_CORES)),
                               trace=_trace)
    global _LAST_RESULTS
    _LAST_RESULTS = res
    out = np.empty((B, C, T), np.float32)
    for b in range(B):
        r0 = res.results[2 * b]["out"].astype(np.float32)
        r1 = res.results[2 * b + 1]["out"].astype(np.float32)
        const = (v_bias_term[2 * b] + v_bias_term[2 * b + 1]
                 + proj_b)[:, None]
        out[b] = (x[b]
                  + r0.transpose(1, 0, 2).reshape(C, T)
                  + r1.transpose(1, 0, 2).reshape(C, T)
                  + const)
    return out


# revision 50
# speedup vs baseline: 1.0821x; 1.0094x over previous
"""AttentionBlock (GroupNorm -> qkv -> softmax attention -> proj + residual)
for Trainium2, 8 NeuronCores, fp8 DoubleRow edition.

Sharding: core = (batch b, head-half hh): each core handles 1 of 4 batches
and 4 of 8 heads, computing a partial projection output; the host sums the
two partials per batch and adds the residual x and proj_b.

Device-side structure (per core):
 - GroupNorm is folded into the weights on the HOST: h = s*x + off with
   per-(batch,channel) s/off from exact f32 stats, so W' = W*diag(s) (fp8)
   and per-out-channel biases ride the PSUM->SBUF drains.  x ships as fp8.
 - ALL matmuls (qkv/scores/av/proj) run in fp8e4 DoubleRow perf mode
   (0.5 cycles/row).  The score matmuls have only a 64-deep contraction
   (per-head channels); DoubleRow needs a k-tile PAIR, so q/k live in
   [128, 2(kt), 4(mc), T] tiles whose kt=1 plane is zero: lhsT/rhs APs
   [64, 2, m] contract over (64 ch + 64 zeros) -- numerically exact, and
   the cost halves.
 - exp(scores) is split between the ACT engine (native Exp) and the DVE
   (a custom quartic c2*(c0*x+c1)^4 DVE op registered at import time).
 - softmax normalization: rowsums come free via a ones-column in v^T; per
   unit ONE fused reciprocal [1,2,512] (DVE), ONE partition_broadcast
   (GPSIMD) and ONE multiply [64,2,512] (DVE) cover both t-halves.

The mask input is all-True per the problem spec, a numeric no-op.  q/k/GN
biases are folded exactly; v/proj biases are added exactly on the host.
"""

import os
import numpy as np
import ml_dtypes

import concourse.bass as bass
import concourse.tile as tile
from concourse import bacc, mybir, library_config
from concourse.bass_utils import run_bass_kernel_spmd

F32 = mybir.dt.float32
BF16 = mybir.dt.bfloat16
FP8 = mybir.dt.float8e4
AF = mybir.ActivationFunctionType
ALU = mybir.AluOpType
DR = mybir.MatmulPerfMode.DoubleRow
E4 = ml_dtypes.float8_e4m3

B, C, T, H = 4, 512, 2048, 8
CH = 64
G = 32
EPS = 1e-5
HL = 4                 # heads per core
P = 128
TH = T // 2            # 1024, t-half handled per (hd, th)
N_CORES = 8

# fp8 range scales
SW = 64.0              # weight upscale for fp8 (qkv + proj weights)
SQ = 4.0               # q/k sbuf upscale (on top of 1/sqrt(sqrt(ch)))
SV = 64.0              # v sbuf upscale (=SW so vt drain is a plain copy)
GAMMA = 1.0 / (SQ * SQ)  # descale applied inside exp
SCALE = 1.0 / np.sqrt(np.sqrt(CH))

# quartic exp approximation constants (minimax on [-1.7, 1.7])
QC0, QC1, QC2 = 0.24274105, 1.02873227, 1.04374374

# exp engine split: chunk i of 16 per (hd, th) goes to ACT if pattern bit set
EXP_ACT_FRAC = float(os.environ.get("EXP_ACT_FRAC", "0.595"))
# drain engine choices ("act" or "dve")
QK_DRAIN = os.environ.get("QK_DRAIN", "alt")
VT_DRAIN = os.environ.get("VT_DRAIN", "act")
PJ_DRAIN = os.environ.get("PJ_DRAIN", "alt")
# tail proj blocks drain on ACT only, so the last unit's normalize chain
# never queues behind a proj drain on the in-order DVE
PJT_TAIL_ACT = int(os.environ.get("PJT_TAIL_ACT", "1"))

# split the very last proj block's drains into ap-512 halves on both
# engines so the final out-DMA starts sooner
PJT_SPLIT_LAST = int(os.environ.get("PJT_SPLIT_LAST", "0"))
# final proj block: first drain on DVE (frees before ACT at the tail)
PJT3_DVE = int(os.environ.get("PJT3_DVE", "1"))
LAST_PAIR_SPLIT = int(os.environ.get("LAST_PAIR_SPLIT", "0"))
VT_POS = os.environ.get("VT_POS", "spread")
WPOOL = int(os.environ.get("WPOOL", "12"))
VT_SLOTS = tuple(int(v) for v in os.environ.get("VT_SLOTS", "1,3,5,7").split(","))
WARMUP = int(os.environ.get("WARMUP", "12"))
NORM_LAG = int(os.environ.get("NORM_LAG", "0"))
SPLIT_LAST = int(os.environ.get("SPLIT_LAST", "0"))
OUTP = int(os.environ.get("OUTP", "6"))
# PJ_POOL: where proj psum tiles come from. 0 = borrow the sps ring
# (stalls the score stream ~1us per burst); 2 = borrow the sh (avs) ring
# (proj naturally waits for the freshly-freed avs slot).  1 = dedicated
# pool, only with SPS_BUFS=2 — measured much worse, keep for reference.
PJ_POOL = int(os.environ.get("PJ_POOL", "2"))
SPS_BUFS = int(os.environ.get("SPS_BUFS", "2" if PJ_POOL == 1 else "3"))

# ---- custom DVE op: EXP4_ANT = c2*(c0*x+c1)^4 ------------------------------
from concourse import dve_ops as _dops
from concourse.dve_spec import Spec as _Spec, Src0 as _Src0, C0 as _C0, \
    C1 as _C1, C2 as _C2, sq as _sq, lower as _lower
from concourse.dve_uop import DveOpSpec as _DveOpSpec


def _exp4_ref(in0, in1, c0, c1, c2):
    y = np.square(np.square(in0.astype(np.float32) * c0 + c1)) * c2
    return y.astype(np.float32)


def _register_exp4():
    for op in _dops.OPS:
        if op.name == "EXP4_ANT":
            return op
    spec = _Spec(body=_sq(_sq(_Src0 * _C0 + _C1)) * _C2, reference=_exp4_ref)
    shas = {}
    for ver in ("v3", "v4"):
        s = _DveOpSpec(name="EXP4_ANT", opcode=0, uops=_lower(spec, ver=ver),
                       rd1_en=False)
        shas[ver] = s.sha(ver)
    op = _dops.DveOp("EXP4_ANT", spec, subdim=False, uops_sha=shas)
    _dops.OPS.append(op)
    _dops.CUSTOM_DVE_SPECS[op.name] = spec
    _dops._SUB_OPCODE_FOR_NAME[op.name] = (
        max(_dops._SUB_OPCODE_FOR_NAME.values()) + 1)
    return op


EXP4 = _register_exp4()


EXP_ACT_EARLY = float(os.environ.get("EXP_ACT_EARLY", "0.50"))
EXP_EARLY_CHUNKS = int(os.environ.get("EXP_EARLY_CHUNKS", "24"))
EXP_ACT_LATE = float(os.environ.get("EXP_ACT_LATE", "0.55"))
EXP_LATE_CHUNKS = int(os.environ.get("EXP_LATE_CHUNKS", "16"))


def _exp_engine_pattern():
    """One entry per exp chunk (128 total): True -> ACT, False -> DVE.
    Early chunks lean DVE (ACT busy with qkv drains); late chunks move
    toward 50/50 so both engines finish the last unit together."""
    if os.environ.get("PATTERN", "") == "unit":
        # unit-position-aware: each unit's norm (recip+mult) lands on DVE
        # while the NEXT unit's chunks 4..11 stream, so give ACT a larger
        # share there
        hi = float(os.environ.get("PAT_HI", "0.75"))
        lo = 2 * EXP_ACT_FRAC - hi
        pat = []
        acc = 0.0
        for i in range(128):
            pos = i % 16
            f = hi if 4 <= pos < 12 else lo
            acc += f
            if acc >= 1.0 - 1e-9:
                acc -= 1.0
                pat.append(True)
            else:
                pat.append(False)
        return pat
    total_act = EXP_ACT_FRAC * 128
    early_act = EXP_ACT_EARLY * EXP_EARLY_CHUNKS
    late_act = EXP_ACT_LATE * EXP_LATE_CHUNKS
    mid = 128 - EXP_EARLY_CHUNKS - EXP_LATE_CHUNKS
    mid_frac = (total_act - early_act - late_act) / mid
    pat = []
    acc = 0.0
    for i in range(128):
        if i < EXP_EARLY_CHUNKS:
            f = EXP_ACT_EARLY
        elif i >= 128 - EXP_LATE_CHUNKS:
            f = EXP_ACT_LATE
        else:
            f = mid_frac
        acc += f
        if acc >= 1.0 - 1e-9:
            acc -= 1.0
            pat.append(True)
        else:
            pat.append(False)
    return pat


def _build_nc():
    nc = bacc.Bacc(
        "TRN2",
        target_bir_lowering=False,
        debug=False,
        enable_asserts=False,
        num_devices=N_CORES,
    )
    x_d = nc.dram_tensor("x", [P, 4, T], FP8, kind="ExternalInput").ap()
    wqk_d = nc.dram_tensor("wqk", [P, 4, 4, P], FP8, kind="ExternalInput").ap()
    wv_d = nc.dram_tensor("wv", [P, 4, 2 * P], FP8, kind="ExternalInput").ap()
    wp_d = nc.dram_tensor("wp", [P, 2, 4, P], FP8, kind="ExternalInput").ap()
    cqk_d = nc.dram_tensor("cqk", [P, 4], F32, kind="ExternalInput").ap()
    out_d = nc.dram_tensor("out", [P, 4, T], BF16, kind="ExternalOutput").ap()

    pat = _exp_engine_pattern()

    with tile.TileContext(nc) as tc:
        with (
            tc.tile_pool(name="consts", bufs=1) as consts,
            tc.tile_pool(name="xp", bufs=1) as xp,
            tc.tile_pool(name="qkp", bufs=1) as qkp,
            tc.tile_pool(name="vtp", bufs=1) as vtp,
            tc.tile_pool(name="ap", bufs=1) as apool,
            tc.tile_pool(name="wpool", bufs=WPOOL) as wpool,
            tc.tile_pool(name="rhop", bufs=3) as rhop,
            tc.tile_pool(name="repp", bufs=3) as repp,
            tc.tile_pool(name="outp", bufs=OUTP) as outp,
            tc.tile_pool(name="ps_sps", bufs=SPS_BUFS, space="PSUM") as ps_sps,
            tc.tile_pool(name="ps_pj", bufs=1, space="PSUM") as ps_pj,
            tc.tile_pool(name="ps_sh", bufs=1, space="PSUM") as ps_sh,
        ):
            nc.gpsimd.load_library(library_config.attn)

            # ---- DMA in ----
            # DMA_GP=1: input DMAs issue from the GPSIMD queue (Pool DMA
            # config is 25ns/instr vs SP's 565) and x splits into quarters
            # so the 16 parallel DMA engines land it sooner.
            dma_eng = nc.gpsimd if int(os.environ.get("DMA_GP", "0")) \
                else nc.sync
            # DMA transfers are SERIAL on the DMA device: cqk (56ns) must
            # not queue behind the 1.5us x second half -- the first qk
            # drains wait on it.
            x_sb = xp.tile([P, 4, T], FP8)
            wqk = consts.tile([P, 4, 4, P], FP8)
            cqk = consts.tile([P, 4], F32)
            wv = consts.tile([P, 4, 2 * P], FP8)
            wp = consts.tile([P, 2, 4, P], FP8)
            if int(os.environ.get("DMA_MIX", "0")):
                # spread input DMAs over DIFFERENT engines' DGE queues so
                # the transfers overlap on the 16-engine DMA device instead
                # of serializing on one queue
                nc.sync.dma_start(x_sb[:, :, 0:TH], x_d[:, :, 0:TH])
                nc.gpsimd.dma_start(wqk, wqk_d)
                nc.gpsimd.dma_start(cqk, cqk_d)
                nc.scalar.dma_start(x_sb[:, :, TH:T], x_d[:, :, TH:T])
                nc.gpsimd.dma_start(wv, wv_d)
                nc.gpsimd.dma_start(wp, wp_d)
            else:
                dma_eng.dma_start(x_sb[:, :, 0:TH], x_d[:, :, 0:TH])
                dma_eng.dma_start(wqk, wqk_d)
                dma_eng.dma_start(cqk, cqk_d)
                dma_eng.dma_start(x_sb[:, :, TH:T], x_d[:, :, TH:T])
                dma_eng.dma_start(wv, wv_d)
                dma_eng.dma_start(wp, wp_d)

            # PE p-state warmup while input DMAs land: dummy matmuls on a
            # const tile keep the PE continuously busy so real matmuls start
            # at full clock.
            warm = consts.tile([P, P], FP8)
            nc.vector.memset(warm, 0.0)
            warm2 = consts.tile([P, 512], FP8)
            nc.vector.memset(warm2, 0.0)
            warm_ps = ps_sps.tile([P, 512], F32, tag="sps", name="warm")
            for _ in range(WARMUP):
                nc.tensor.matmul(warm_ps[:, 0:128], lhsT=warm,
                                 rhs=warm2[:, 0:128], start=True, stop=True)
            # a few more dummies that READ x_sb: they wait on the x first
            # half landing, so the PE stays warm right up to the first
            # real qk matmul instead of dropping out of p-state
            for _ in range(int(os.environ.get("WARM2", "0"))):
                nc.tensor.matmul(warm_ps[:, 0:512], lhsT=warm,
                                 rhs=x_sb[:, 0, 0:512], start=True, stop=True)

            # ---- qk matmuls + drains ----
            # qk_sb: [128, 2 (kt), 4 (mc), T] fp8.  kt=0 holds q/k data
            # (mc 0: q heads 0/1, 1: q heads 2/3, 2: k heads 0/1, 3: k
            # heads 2/3; head parity is the 64-partition band), kt=1 is
            # ZERO so score matmuls can run DoubleRow with APs
            # [64, 2(kt), m] -- contraction (64 ch + 64 zeros).
            qk_sb = qkp.tile([P, 2, 4, T], FP8)
            # zero the kt=1 planes on the (otherwise idle) GPSIMD engine,
            # in first-use order: k_a, q_a, k_b, q_b
            for mc in (2, 0, 3, 1):
                nc.gpsimd.memset(qk_sb[:, 1, mc, :], 0.0)

            def qk_group(mc, tc2):
                # fused [128, 1024] tile (two tc4 halves) in the sps pool
                qkt = ps_sps.tile([P, 2, 512], F32, tag="sps",
                                  name=f"qk{mc}{tc2}")
                for t2 in range(2):
                    tc4 = tc2 * 2 + t2
                    for kcp in range(2):
                        nc.tensor.matmul(
                            qkt[:, t2, :],
                            lhsT=wqk[:, 2 * kcp : 2 * kcp + 2, mc, :],
                            rhs=x_sb[:, 2 * kcp : 2 * kcp + 2,
                                     tc4 * 512 : (tc4 + 1) * 512],
                            start=(kcp == 0), stop=(kcp == 1),
                            perf_mode=DR,
                        )
                dst = qk_sb[:, 0, mc, tc2 * 1024 : (tc2 + 1) * 1024]
                if QK_DRAIN == "act" or (QK_DRAIN == "alt" and mc in (0, 1)) \
                        or (QK_DRAIN == "alt2" and mc in (2, 3)):
                    nc.scalar.activation(
                        dst,
                        qkt.rearrange("p a b -> p (a b)"),
                        AF.Identity,
                        bias=cqk[:, mc : mc + 1],
                        scale=float(SCALE * SQ / SW),
                    )
                else:
                    nc.vector.tensor_scalar(
                        dst,
                        qkt.rearrange("p a b -> p (a b)"),
                        float(SCALE * SQ / SW),
                        cqk[:, mc : mc + 1],
                        ALU.mult, ALU.add,
                    )

            # ---- vt matmuls + drains ----
            # vt_sb: [128 (s%128), 16 (sc), 4 (hd), 128] fp8; cols 64..127
            # are ONES so av rows 64..127 all come out as the rowsum -- a
            # 64-way replicated rowsum that feeds reciprocal directly (no
            # partition_broadcast needed).
            vt_sb = vtp.tile([P, 16, HL, 2 * CH], FP8)
            nc.gpsimd.memset(vt_sb[:, :, :, CH : 2 * CH], 1.0)

            def vt_group4(g):
                # fused tile: 4 sc chunks (= scp pair 2g, 2g+1)
                vtt = ps_sps.tile([P, 4, 2 * P], F32, tag="sps",
                                  name=f"vt{g}")
                for s4 in range(4):
                    sc = g * 4 + s4
                    for kcp in range(2):
                        nc.tensor.matmul(
                            vtt[:, s4, :],
                            lhsT=x_sb[:, 2 * kcp : 2 * kcp + 2,
                                      sc * P : (sc + 1) * P],
                            rhs=wv[:, 2 * kcp : 2 * kcp + 2, :],
                            start=(kcp == 0), stop=(kcp == 1),
                            perf_mode=DR,
                        )
                if VT_DRAIN == "act" or (VT_DRAIN == "alt" and g % 2 == 0):
                    nc.scalar.activation(
                        vt_sb[:, 4 * g : 4 * g + 4, :, 0:CH],
                        vtt.rearrange("p a (h c) -> p a h c", h=HL),
                        AF.Identity,
                    )
                else:
                    nc.vector.tensor_copy(
                        vt_sb[:, 4 * g : 4 * g + 4, :, 0:CH],
                        vtt.rearrange("p a (h c) -> p a h c", h=HL),
                    )

            # qk for the FIRST chunks only (k_a/q_a tc2=0); the remaining
            # 6 qk groups and the 4 vt groups are injected INTO the
            # attention stream (QK_STREAM=1) so the first score matmul
            # fires as soon as the first x half + wqk land, instead of
            # after all 32 qk matmuls.
            QK_STREAM = int(os.environ.get("QK_STREAM", "0"))
            qk_group(2, 0)                 # k_a s 0..1024
            qk_group(0, 0)                 # q_a t 0..1024
            if QK_STREAM:
                stream_extras = {
                    0: lambda: qk_group(2, 1),   # k_a s 1024.. (need g4)
                    1: lambda: vt_group4(0),     # sc 0-3      (need g5)
                    2: lambda: qk_group(3, 0),   # k_b         (need g16)
                    3: lambda: qk_group(1, 0),   # q_b         (need g16)
                    4: lambda: vt_group4(1),     # sc 4-7      (need g7)
                    5: lambda: qk_group(0, 1),   # q_a th1     (need g32)
                    6: lambda: vt_group4(2),     # sc 8-11     (need g9)
                    7: lambda: vt_group4(3),     # sc 12-15    (need g11)
                    8: lambda: qk_group(3, 1),   # k_b s 1024..(need g20)
                    9: lambda: qk_group(1, 1),   # q_b th1     (need g48)
                }
            else:
                qk_group(2, 1)
                qk_group(0, 1)
                for tc2 in range(2):
                    qk_group(3, tc2)
                    qk_group(1, tc2)
                stream_extras = {}
                if VT_POS == "pre":
                    for g in range(4):
                        vt_group4(g)

            # ---- attention ----
            a_sb = apool.tile([P, 2, T], FP8)

            # attention as a software-pipelined chunk stream: av matmuls
            # lag the scores/exp stream by AV_LAG chunk-pairs so PE never
            # waits on the previous unit's last exp at unit boundaries.
            AV_LAG = int(os.environ.get("AV_LAG", "6"))
            units = [(hd, th) for th in range(2) for hd in range(HL)]
            state = {}   # u -> dict(avs, w_ts)
            exp_ctr = [0]

            def unit_geom(u):
                hd, th = units[u]
                b0 = 64 * (hd % 2)
                q_mc = 0 if hd < 2 else 1
                k_mc = 2 if hd < 2 else 3
                return hd, th, b0, q_mc, k_mc

            def emit_chunk(u, scp):
                hd, th, b0, q_mc, k_mc = unit_geom(u)
                toff = th * TH
                if scp == 0:
                    state[u] = dict(
                        avs=ps_sh.tile([P, 2, 512], F32, tag="sh",
                                       name=f"av{hd}{th}"),
                        w_ts={})
                w_t = wpool.tile([P, 2, TH], FP8, name="wt")
                state[u]["w_ts"][scp] = w_t
                split = u >= len(units) - SPLIT_LAST
                for j in range(2):
                    sc = scp * 2 + j
                    sps = ps_sps.tile([P, TH], F32, tag="sps", name="sps")
                    for tq in range(2):
                        nc.tensor.matmul(
                            sps[:, tq * 512 : (tq + 1) * 512],
                            lhsT=qk_sb[b0 : b0 + CH, :, k_mc,
                                       sc * P : (sc + 1) * P],
                            rhs=qk_sb[b0 : b0 + CH, :, q_mc,
                                      toff + tq * 512 : toff + (tq + 1) * 512],
                            start=True, stop=True,
                            perf_mode=DR,
                        )
                    if split:
                        # tail units: halve each chunk across BOTH engines so
                        # the slot frees sooner and av-tq halves unblock early
                        nc.scalar.activation(
                            w_t[:, j, 0:512], sps[:, 0:512], AF.Exp,
                            scale=float(GAMMA))
                        nc.vector._custom_dve(
                            EXP4, out=w_t[:, j, 512:TH], in0=sps[:, 512:TH],
                            s0=float(QC0 * GAMMA), s1=float(QC1),
                            imm2=float(QC2))
                    elif (LAST_PAIR_SPLIT and u == len(units) - 1
                          and scp == 7):
                        # force the final chunk-pair onto BOTH engines so
                        # the last av fires as early as possible
                        if j == 0:
                            nc.scalar.activation(
                                w_t[:, j, :], sps, AF.Exp, scale=float(GAMMA))
                        else:
                            nc.vector._custom_dve(
                                EXP4, out=w_t[:, j, :], in0=sps,
                                s0=float(QC0 * GAMMA), s1=float(QC1),
                                imm2=float(QC2))
                    elif pat[exp_ctr[0]]:
                        nc.scalar.activation(
                            w_t[:, j, :], sps, AF.Exp, scale=float(GAMMA))
                    else:
                        nc.vector._custom_dve(
                            EXP4, out=w_t[:, j, :], in0=sps,
                            s0=float(QC0 * GAMMA), s1=float(QC1),
                            imm2=float(QC2))
                    exp_ctr[0] += 1

            def emit_av(u, scp):
                hd, th, b0, q_mc, k_mc = unit_geom(u)
                avs = state[u]["avs"]
                w_t = state[u]["w_ts"].pop(scp)
                for tq in range(2):
                    nc.tensor.matmul(
                        avs[:, tq, :],
                        lhsT=vt_sb[:, 2 * scp : 2 * scp + 2, hd, :],
                        rhs=w_t[:, :, tq * 512 : (tq + 1) * 512],
                        start=(scp == 0), stop=(scp == 7),
                        perf_mode=DR,
                    )

            def emit_normalize(u, between=None):
                hd, th, b0, q_mc, k_mc = unit_geom(u)
                toff = th * TH
                avs = state[u]["avs"]
                if between is None:
                    # fused across both tq halves: reciprocal of the 64
                    # replicated rowsum rows IS the broadcast recip; then
                    # one multiply [64,2,512]
                    rep = repp.tile([CH, 2, 512], F32, name="rep")
                    nc.vector.reciprocal(rep, avs[CH : 2 * CH, :, :])
                    nc.vector.tensor_tensor(
                        a_sb[CH * (hd % 2) : CH * (hd % 2) + CH, hd // 2,
                             toff : toff + TH],
                        avs[0:CH, :, :], rep, ALU.mult,
                    )
                else:
                    # last unit: split per tq so proj_tc(2) can interleave
                    for tq in range(2):
                        rep = repp.tile([CH, 512], F32, name="rep")
                        nc.vector.reciprocal(rep, avs[CH : 2 * CH, tq, :])
                        nc.vector.tensor_tensor(
                            a_sb[CH * (hd % 2) : CH * (hd % 2) + CH, hd // 2,
                                 toff + tq * 512 : toff + (tq + 1) * 512],
                            avs[0:CH, tq, :], rep, ALU.mult,
                        )
                        if tq == 0:
                            between()
                del state[u]

            def run_attention(extra=()):
                stream = [(u, scp) for u in range(len(units))
                          for scp in range(8)]
                norm_q = []   # units whose avs are done, normalize deferred

                def pop_norm():
                    lu = norm_q.pop(0)
                    emit_normalize(lu)
                    if lu == 3:           # last th0 unit done
                        proj_tc(0)
                    elif lu == 5:
                        proj_tc(1)

                for g, (u, scp) in enumerate(stream):
                    emit_chunk(u, scp)
                    if QK_STREAM:
                        if g in stream_extras:
                            stream_extras[g]()
                    elif VT_POS == "stream" and g < 4:
                        vt_group4(g)
                    elif VT_POS == "spread" and g in VT_SLOTS:
                        vt_group4(VT_SLOTS.index(g))
                    lag = g - AV_LAG
                    if lag >= 0:
                        lu, lscp = stream[lag]
                        emit_av(lu, lscp)
                        if lscp == 7:
                            norm_q.append(lu)
                    if norm_q:
                        lu = norm_q[0]
                        close_g = (lu * 8 + 7) + AV_LAG  # g when avs closed
                        if g >= close_g + NORM_LAG:
                            pop_norm()
                for lu, lscp in stream[-AV_LAG:]:
                    emit_av(lu, lscp)
                    if lscp == 7:
                        norm_q.append(lu)
                while len(norm_q) > 1:
                    pop_norm()
                emit_normalize(norm_q.pop(0),
                               between=lambda: proj_tc(2, tail=True))

            def proj_tc(tc4, tail=False):
                # oc-PAIR tiles with one fused ap-1024 drain each.  Tail
                # blocks (tc4 2,3) borrow the sps ring -- the score stream
                # is over, so its slots are free; early blocks follow
                # PJ_POOL (default: sh ring, whose slot just freed).
                for op2 in range(2):
                    if tail or PJ_POOL == 0:
                        pjt = ps_sps.tile([P, 2, 512], F32, tag="sps",
                                          name=f"pjs{tc4}{op2}")
                    elif PJ_POOL == 1:
                        pjt = ps_pj.tile([P, 2, 512], F32, tag="pj",
                                         name=f"pjs{tc4}{op2}")
                    else:
                        pjt = ps_sh.tile([P, 2, 512], F32, tag="sh",
                                         name=f"pjs{tc4}{op2}")
                    for o2 in range(2):
                        oc = op2 * 2 + o2
                        nc.tensor.matmul(
                            pjt[:, o2, :],
                            lhsT=wp[:, :, oc, :],
                            rhs=a_sb[:, :, tc4 * 512 : (tc4 + 1) * 512],
                            start=True, stop=True,
                            perf_mode=DR,
                        )
                    ot = outp.tile([P, 2, 512], BF16, name="otp")
                    if tc4 == 3 and tail and PJT_SPLIT_LAST:
                        # final block: halve each drain across both engines
                        # and ship each half as its own DMA so the last
                        # transfer starts as early as possible
                        for o2, eng in ((0, "act"), (1, "dve")):
                            if eng == "act":
                                nc.scalar.activation(
                                    ot[:, o2, :], pjt[:, o2, :], AF.Identity,
                                    scale=float(1.0 / (SV * SW)))
                            else:
                                nc.vector.tensor_scalar(
                                    ot[:, o2, :], pjt[:, o2, :],
                                    float(1.0 / (SV * SW)), None, ALU.mult)
                            nc.sync.dma_start(
                                out_d[:, 2 * op2 + o2,
                                      tc4 * 512 : (tc4 + 1) * 512],
                                ot[:, o2, :])
                        continue
                    if tc4 == 3 and tail and PJT3_DVE:
                        # final block: DVE frees first (after the norm
                        # chain); ACT is still on proj2's drains
                        on_act = op2 == 1
                    else:
                        on_act = (tail and PJT_TAIL_ACT) \
                            or PJ_DRAIN == "act" \
                            or (PJ_DRAIN == "alt" and op2 == 0)
                    if on_act:
                        nc.scalar.activation(
                            ot, pjt, AF.Identity,
                            scale=float(1.0 / (SV * SW)))
                    else:
                        nc.vector.tensor_scalar(
                            ot, pjt, float(1.0 / (SV * SW)),
                            None, ALU.mult)
                    nc.sync.dma_start(
                        out_d[:, 2 * op2 : 2 * op2 + 2,
                              tc4 * 512 : (tc4 + 1) * 512], ot)

            run_attention()
            proj_tc(3, tail=True)
    nc.compile()
    return nc


_NC = None
_LAST_RESULTS = None


def _get_nc():
    global _NC
    if _NC is None:
        _NC = _build_nc()
    return _NC


def _fp8(a):
    return np.ascontiguousarray(a.astype(np.float32).astype(E4))


def kernel(x, mask, gn_gamma, gn_beta, qkv_w, qkv_b, proj_w, proj_b,
           _trace=False):
    del mask  # all-True per problem spec
    x = np.asarray(x, np.float32)
    gn_gamma = np.asarray(gn_gamma, np.float32)
    gn_beta = np.asarray(gn_beta, np.float32)
    qkv_w = np.asarray(qkv_w, np.float32)
    qkv_b = np.asarray(qkv_b, np.float32)
    proj_w = np.asarray(proj_w, np.float32)
    proj_b = np.asarray(proj_b, np.float32)

    # exact GroupNorm stats per batch (host, f32)
    xg = x.reshape(B, G, C // G, T)
    mu = xg.mean(axis=(2, 3))                      # [B, G]
    var = xg.var(axis=(2, 3))                      # [B, G]
    s_bg = 1.0 / np.sqrt(var + EPS)                # [B, G]
    s_bc = np.repeat(s_bg, C // G, axis=1) * gn_gamma[None, :]      # [B, C]
    off_bc = gn_beta[None, :] - np.repeat(mu * s_bg, C // G, axis=1) \
        * gn_gamma[None, :]                        # [B, C]

    in_maps = []
    v_bias_term = {}
    for core in range(N_CORES):
        b, hh = core // 2, core % 2
        heads = [hh * HL + i for i in range(HL)]
        # column order for q/k: [head][ch]; mc blocks = head pairs
        q_rows = np.concatenate(
            [np.arange(h * 192, h * 192 + 64) for h in heads])
        k_rows = q_rows + 64
        v_rows = np.concatenate([np.arange(h * 192 + 128, h * 192 + 192)
                                 for h in heads])

        s = s_bc[b]                                # [C]
        off = off_bc[b]                            # [C]

        wq = qkv_w[q_rows] * s[None, :]            # [256, 512]
        wk = qkv_w[k_rows] * s[None, :]
        wv_ = qkv_w[v_rows] * s[None, :]
        # wqk dram layout [p(c%128), kc(c//128), mc, m(128)]
        wqk_m = np.concatenate([wq, wk], 0)        # [512(m), 512(c)]
        wqk_t = (wqk_m.T.reshape(4, P, 4, P)
                 .transpose(1, 0, 2, 3))           # [p, kc, mc, m]
        wqk_t = wqk_t * SW
        wv_t = wv_.T.reshape(4, P, 2 * P).transpose(1, 0, 2) * SW
        # proj columns for this half, reordered to head-band x ch
        wp_cols = proj_w[:, [hh * 256 + i for i in range(256)]]  # [512, 256]
        # a_sb rows: [hd%2 band (64), hd//2 ktile]: channel (hd, ch) sits at
        # row 64*(hd%2)+ch of ktile hd//2 -> input index hd*64+ch
        perm = np.array([(kt * 2 + band) * 64 + ch
                         for kt in range(2) for band in range(2)
                         for ch in range(64)])
        # rows of wp lhsT tile [p, kt, oc, m]: p = 64*band+ch
        wp_in = wp_cols[:, perm]                   # [512 out, 256 perm-in]
        wp_t = (wp_in.T.reshape(2, P, 4, P)
                .transpose(1, 0, 2, 3)) * SW       # [p, kt, oc, m]

        cq = (qkv_w[q_rows] @ off + qkv_b[q_rows]) * SCALE * SQ
        ck = (qkv_w[k_rows] @ off + qkv_b[k_rows]) * SCALE * SQ
        cqk = np.stack([cq[:P], cq[P:], ck[:P], ck[P:]], axis=1)  # [128, 4]

        x_t = x[b].reshape(4, P, T).transpose(1, 0, 2)

        in_maps.append(dict(
            x=_fp8(x_t),
            wqk=_fp8(wqk_t),
            wv=_fp8(wv_t),
            wp=_fp8(wp_t),
            cqk=np.ascontiguousarray(cqk, dtype=np.float32),
        ))
        # v bias + GN-offset contribution through v, exact on host:
        cv = qkv_w[v_rows] @ off + qkv_b[v_rows]   # [256]
        v_bias_term[core] = proj_w[:, hh * 256 : hh * 256 + 256] @ cv  # [512]

    nc = _get_nc()
    res = run_bass_kernel_spmd(nc, in_maps, core_ids=list(range(N_CORES)),
                               trace=_trace)
    global _LAST_RESULTS
    _LAST_RESULTS = res
    out = np.empty((B, C, T), np.float32)
    for b in range(B):
        r0 = res.results[2 * b]["out"].astype(np.float32)
        r1 = res.results[2 * b + 1]["out"].astype(np.float32)
        const = (v_bias_term[2 * b] + v_bias_term[2 * b + 1]
                 + proj_b)[:, None]
        out[b] = (x[b]
                  + r0.transpose(1, 0, 2).reshape(C, T)
                  + r1.transpose(1, 0, 2).reshape(C, T)
                  + const)
    return out


# revision 52
# speedup vs baseline: 1.0911x; 1.0083x over previous
"""AttentionBlock (GroupNorm -> qkv -> softmax attention -> proj + residual)
for Trainium2, 8 NeuronCores, fp8 DoubleRow edition.

Sharding: core = (batch b, head-half hh): each core handles 1 of 4 batches
and 4 of 8 heads, computing a partial projection output; the host sums the
two partials per batch and adds the residual x and proj_b.

Device-side structure (per core):
 - GroupNorm is folded into the weights on the HOST: h = s*x + off with
   per-(batch,channel) s/off from exact f32 stats, so W' = W*diag(s) (fp8)
   and per-out-channel biases ride the PSUM->SBUF drains.  x ships as fp8.
 - ALL matmuls (qkv/scores/av/proj) run in fp8e4 DoubleRow perf mode
   (0.5 cycles/row).  The score matmuls have only a 64-deep contraction
   (per-head channels); DoubleRow needs a k-tile PAIR, so q/k live in
   [128, 2(kt), 4(mc), T] tiles whose kt=1 plane is zero: lhsT/rhs APs
   [64, 2, m] contract over (64 ch + 64 zeros) -- numerically exact, and
   the cost halves.
 - exp(scores) is split between the ACT engine (native Exp) and the DVE
   (a custom quartic c2*(c0*x+c1)^4 DVE op registered at import time).
 - softmax normalization: rowsums come free via a ones-column in v^T; per
   unit ONE fused reciprocal [1,2,512] (DVE), ONE partition_broadcast
   (GPSIMD) and ONE multiply [64,2,512] (DVE) cover both t-halves.

The mask input is all-True per the problem spec, a numeric no-op.  q/k/GN
biases are folded exactly; v/proj biases are added exactly on the host.
"""

import os
import numpy as np
import ml_dtypes

import concourse.bass as bass
import concourse.tile as tile
from concourse import bacc, mybir, library_config
from concourse.bass_utils import run_bass_kernel_spmd

F32 = mybir.dt.float32
BF16 = mybir.dt.bfloat16
FP8 = mybir.dt.float8e4
AF = mybir.ActivationFunctionType
ALU = mybir.AluOpType
DR = mybir.MatmulPerfMode.DoubleRow
E4 = ml_dtypes.float8_e4m3

B, C, T, H = 4, 512, 2048, 8
CH = 64
G = 32
EPS = 1e-5
HL = 4                 # heads per core
P = 128
TH = T // 2            # 1024, t-half handled per (hd, th)
N_CORES = 8

# fp8 range scales
SW = 64.0              # weight upscale for fp8 (qkv + proj weights)
SQ = 4.0               # q/k sbuf upscale (on top of 1/sqrt(sqrt(ch)))
SV = 64.0              # v sbuf upscale (=SW so vt drain is a plain copy)
GAMMA = 1.0 / (SQ * SQ)  # descale applied inside exp
SCALE = 1.0 / np.sqrt(np.sqrt(CH))

# quartic exp approximation constants (minimax on [-1.7, 1.7])
QC0, QC1, QC2 = 0.24274105, 1.02873227, 1.04374374

# exp engine split: chunk i of 16 per (hd, th) goes to ACT if pattern bit set
EXP_ACT_FRAC = float(os.environ.get("EXP_ACT_FRAC", "0.595"))
# drain engine choices ("act" or "dve")
QK_DRAIN = os.environ.get("QK_DRAIN", "alt")
VT_DRAIN = os.environ.get("VT_DRAIN", "act")
PJ_DRAIN = os.environ.get("PJ_DRAIN", "alt")
# tail proj blocks drain on ACT only, so the last unit's normalize chain
# never queues behind a proj drain on the in-order DVE
PJT_TAIL_ACT = int(os.environ.get("PJT_TAIL_ACT", "1"))

# split the very last proj block's drains into ap-512 halves on both
# engines so the final out-DMA starts sooner
PJT_SPLIT_LAST = int(os.environ.get("PJT_SPLIT_LAST", "0"))
# final proj block: first drain on DVE (frees before ACT at the tail)
PJT3_DVE = int(os.environ.get("PJT3_DVE", "1"))
LAST_PAIR_SPLIT = int(os.environ.get("LAST_PAIR_SPLIT", "0"))
VT_POS = os.environ.get("VT_POS", "spread")
WPOOL = int(os.environ.get("WPOOL", "64"))
VT_SLOTS = tuple(int(v) for v in os.environ.get("VT_SLOTS", "1,3,5,7").split(","))
WARMUP = int(os.environ.get("WARMUP", "12"))
NORM_LAG = int(os.environ.get("NORM_LAG", "0"))
SPLIT_LAST = int(os.environ.get("SPLIT_LAST", "0"))
OUTP = int(os.environ.get("OUTP", "6"))
# PJ_POOL: where proj psum tiles come from. 0 = borrow the sps ring
# (stalls the score stream ~1us per burst); 2 = borrow the sh (avs) ring
# (proj naturally waits for the freshly-freed avs slot).  1 = dedicated
# pool, only with SPS_BUFS=2 — measured much worse, keep for reference.
PJ_POOL = int(os.environ.get("PJ_POOL", "2"))
SPS_BUFS = int(os.environ.get("SPS_BUFS", "2" if PJ_POOL == 1 else "3"))

# ---- custom DVE op: EXP4_ANT = c2*(c0*x+c1)^4 ------------------------------
from concourse import dve_ops as _dops
from concourse.dve_spec import Spec as _Spec, Src0 as _Src0, C0 as _C0, \
    C1 as _C1, C2 as _C2, sq as _sq, lower as _lower
from concourse.dve_uop import DveOpSpec as _DveOpSpec


def _exp4_ref(in0, in1, c0, c1, c2):
    y = np.square(np.square(in0.astype(np.float32) * c0 + c1)) * c2
    return y.astype(np.float32)


def _register_exp4():
    for op in _dops.OPS:
        if op.name == "EXP4_ANT":
            return op
    spec = _Spec(body=_sq(_sq(_Src0 * _C0 + _C1)) * _C2, reference=_exp4_ref)
    shas = {}
    for ver in ("v3", "v4"):
        s = _DveOpSpec(name="EXP4_ANT", opcode=0, uops=_lower(spec, ver=ver),
                       rd1_en=False)
        shas[ver] = s.sha(ver)
    op = _dops.DveOp("EXP4_ANT", spec, subdim=False, uops_sha=shas)
    _dops.OPS.append(op)
    _dops.CUSTOM_DVE_SPECS[op.name] = spec
    _dops._SUB_OPCODE_FOR_NAME[op.name] = (
        max(_dops._SUB_OPCODE_FOR_NAME.values()) + 1)
    return op


EXP4 = _register_exp4()


EXP_ACT_EARLY = float(os.environ.get("EXP_ACT_EARLY", "0.50"))
EXP_EARLY_CHUNKS = int(os.environ.get("EXP_EARLY_CHUNKS", "24"))
EXP_ACT_LATE = float(os.environ.get("EXP_ACT_LATE", "0.55"))
EXP_LATE_CHUNKS = int(os.environ.get("EXP_LATE_CHUNKS", "16"))


def _exp_engine_pattern():
    """One entry per exp chunk (128 total): True -> ACT, False -> DVE.
    Early chunks lean DVE (ACT busy with qkv drains); late chunks move
    toward 50/50 so both engines finish the last unit together."""
    if os.environ.get("PATTERN", "") == "unit":
        # unit-position-aware: each unit's norm (recip+mult) lands on DVE
        # while the NEXT unit's chunks 4..11 stream, so give ACT a larger
        # share there
        hi = float(os.environ.get("PAT_HI", "0.75"))
        lo = 2 * EXP_ACT_FRAC - hi
        pat = []
        acc = 0.0
        for i in range(128):
            pos = i % 16
            f = hi if 4 <= pos < 12 else lo
            acc += f
            if acc >= 1.0 - 1e-9:
                acc -= 1.0
                pat.append(True)
            else:
                pat.append(False)
        return pat
    total_act = EXP_ACT_FRAC * 128
    early_act = EXP_ACT_EARLY * EXP_EARLY_CHUNKS
    late_act = EXP_ACT_LATE * EXP_LATE_CHUNKS
    mid = 128 - EXP_EARLY_CHUNKS - EXP_LATE_CHUNKS
    mid_frac = (total_act - early_act - late_act) / mid
    pat = []
    acc = 0.0
    for i in range(128):
        if i < EXP_EARLY_CHUNKS:
            f = EXP_ACT_EARLY
        elif i >= 128 - EXP_LATE_CHUNKS:
            f = EXP_ACT_LATE
        else:
            f = mid_frac
        acc += f
        if acc >= 1.0 - 1e-9:
            acc -= 1.0
            pat.append(True)
        else:
            pat.append(False)
    return pat


def _build_nc():
    nc = bacc.Bacc(
        "TRN2",
        target_bir_lowering=False,
        debug=False,
        enable_asserts=False,
        num_devices=N_CORES,
    )
    x_d = nc.dram_tensor("x", [P, 4, T], FP8, kind="ExternalInput").ap()
    wqk_d = nc.dram_tensor("wqk", [P, 4, 4, P], FP8, kind="ExternalInput").ap()
    wv_d = nc.dram_tensor("wv", [P, 4, 2 * P], FP8, kind="ExternalInput").ap()
    wp_d = nc.dram_tensor("wp", [P, 2, 4, P], FP8, kind="ExternalInput").ap()
    cqk_d = nc.dram_tensor("cqk", [P, 4], F32, kind="ExternalInput").ap()
    out_d = nc.dram_tensor("out", [P, 4, T], BF16, kind="ExternalOutput").ap()

    pat = _exp_engine_pattern()

    with tile.TileContext(nc) as tc:
        with (
            tc.tile_pool(name="consts", bufs=1) as consts,
            tc.tile_pool(name="xp", bufs=1) as xp,
            tc.tile_pool(name="qkp", bufs=1) as qkp,
            tc.tile_pool(name="vtp", bufs=1) as vtp,
            tc.tile_pool(name="ap", bufs=1) as apool,
            tc.tile_pool(name="wpool", bufs=WPOOL) as wpool,
            tc.tile_pool(name="rhop", bufs=3) as rhop,
            tc.tile_pool(name="repp", bufs=3) as repp,
            tc.tile_pool(name="outp", bufs=OUTP) as outp,
            tc.tile_pool(name="ps_sps", bufs=SPS_BUFS, space="PSUM") as ps_sps,
            tc.tile_pool(name="ps_pj", bufs=1, space="PSUM") as ps_pj,
            tc.tile_pool(name="ps_sh", bufs=1, space="PSUM") as ps_sh,
        ):
            nc.gpsimd.load_library(library_config.attn)

            # ---- DMA in ----
            # DMA_GP=1: input DMAs issue from the GPSIMD queue (Pool DMA
            # config is 25ns/instr vs SP's 565) and x splits into quarters
            # so the 16 parallel DMA engines land it sooner.
            dma_eng = nc.gpsimd if int(os.environ.get("DMA_GP", "0")) \
                else nc.sync
            # DMA transfers are SERIAL on the DMA device: cqk (56ns) must
            # not queue behind the 1.5us x second half -- the first qk
            # drains wait on it.
            x_sb = xp.tile([P, 4, T], FP8)
            wqk = consts.tile([P, 4, 4, P], FP8)
            cqk = consts.tile([P, 4], F32)
            wv = consts.tile([P, 4, 2 * P], FP8)
            wp = consts.tile([P, 2, 4, P], FP8)
            if int(os.environ.get("DMA_MIX", "0")):
                # spread input DMAs over DIFFERENT engines' DGE queues so
                # the transfers overlap on the 16-engine DMA device instead
                # of serializing on one queue
                nc.sync.dma_start(x_sb[:, :, 0:TH], x_d[:, :, 0:TH])
                nc.gpsimd.dma_start(wqk, wqk_d)
                nc.gpsimd.dma_start(cqk, cqk_d)
                nc.scalar.dma_start(x_sb[:, :, TH:T], x_d[:, :, TH:T])
                nc.gpsimd.dma_start(wv, wv_d)
                nc.gpsimd.dma_start(wp, wp_d)
            else:
                dma_eng.dma_start(x_sb[:, :, 0:TH], x_d[:, :, 0:TH])
                dma_eng.dma_start(wqk, wqk_d)
                dma_eng.dma_start(cqk, cqk_d)
                dma_eng.dma_start(x_sb[:, :, TH:T], x_d[:, :, TH:T])
                dma_eng.dma_start(wv, wv_d)
                dma_eng.dma_start(wp, wp_d)

            # PE p-state warmup while input DMAs land: dummy matmuls on a
            # const tile keep the PE continuously busy so real matmuls start
            # at full clock.
            warm = consts.tile([P, P], FP8)
            nc.vector.memset(warm, 0.0)
            warm2 = consts.tile([P, 512], FP8)
            nc.vector.memset(warm2, 0.0)
            warm_ps = ps_sps.tile([P, 512], F32, tag="sps", name="warm")
            for _ in range(WARMUP):
                nc.tensor.matmul(warm_ps[:, 0:128], lhsT=warm,
                                 rhs=warm2[:, 0:128], start=True, stop=True)
            # a few more dummies that READ x_sb: they wait on the x first
            # half landing, so the PE stays warm right up to the first
            # real qk matmul instead of dropping out of p-state
            for _ in range(int(os.environ.get("WARM2", "0"))):
                nc.tensor.matmul(warm_ps[:, 0:512], lhsT=warm,
                                 rhs=x_sb[:, 0, 0:512], start=True, stop=True)

            # ---- qk matmuls + drains ----
            # qk_sb: [128, 2 (kt), 4 (mc), T] fp8.  kt=0 holds q/k data
            # (mc 0: q heads 0/1, 1: q heads 2/3, 2: k heads 0/1, 3: k
            # heads 2/3; head parity is the 64-partition band), kt=1 is
            # ZERO so score matmuls can run DoubleRow with APs
            # [64, 2(kt), m] -- contraction (64 ch + 64 zeros).
            qk_sb = qkp.tile([P, 2, 4, T], FP8)
            # zero the kt=1 planes on the (otherwise idle) GPSIMD engine,
            # in first-use order: k_a, q_a, k_b, q_b
            for mc in (2, 0, 3, 1):
                nc.gpsimd.memset(qk_sb[:, 1, mc, :], 0.0)

            def qk_group(mc, tc2):
                # fused [128, 1024] tile (two tc4 halves) in the sps pool
                qkt = ps_sps.tile([P, 2, 512], F32, tag="sps",
                                  name=f"qk{mc}{tc2}")
                for t2 in range(2):
                    tc4 = tc2 * 2 + t2
                    for kcp in range(2):
                        nc.tensor.matmul(
                            qkt[:, t2, :],
                            lhsT=wqk[:, 2 * kcp : 2 * kcp + 2, mc, :],
                            rhs=x_sb[:, 2 * kcp : 2 * kcp + 2,
                                     tc4 * 512 : (tc4 + 1) * 512],
                            start=(kcp == 0), stop=(kcp == 1),
                            perf_mode=DR,
                        )
                dst = qk_sb[:, 0, mc, tc2 * 1024 : (tc2 + 1) * 1024]
                if QK_DRAIN == "act" or (QK_DRAIN == "alt" and mc in (0, 1)) \
                        or (QK_DRAIN == "alt2" and mc in (2, 3)):
                    nc.scalar.activation(
                        dst,
                        qkt.rearrange("p a b -> p (a b)"),
                        AF.Identity,
                        bias=cqk[:, mc : mc + 1],
                        scale=float(SCALE * SQ / SW),
                    )
                else:
                    nc.vector.tensor_scalar(
                        dst,
                        qkt.rearrange("p a b -> p (a b)"),
                        float(SCALE * SQ / SW),
                        cqk[:, mc : mc + 1],
                        ALU.mult, ALU.add,
                    )

            # ---- vt matmuls + drains ----
            # vt_sb: [128 (s%128), 16 (sc), 4 (hd), 128] fp8; cols 64..127
            # are ONES so av rows 64..127 all come out as the rowsum -- a
            # 64-way replicated rowsum that feeds reciprocal directly (no
            # partition_broadcast needed).
            vt_sb = vtp.tile([P, 16, HL, 2 * CH], FP8)
            nc.gpsimd.memset(vt_sb[:, :, :, CH : 2 * CH], 1.0)

            def vt_group4(g):
                # fused tile: 4 sc chunks (= scp pair 2g, 2g+1)
                vtt = ps_sps.tile([P, 4, 2 * P], F32, tag="sps",
                                  name=f"vt{g}")
                for s4 in range(4):
                    sc = g * 4 + s4
                    for kcp in range(2):
                        nc.tensor.matmul(
                            vtt[:, s4, :],
                            lhsT=x_sb[:, 2 * kcp : 2 * kcp + 2,
                                      sc * P : (sc + 1) * P],
                            rhs=wv[:, 2 * kcp : 2 * kcp + 2, :],
                            start=(kcp == 0), stop=(kcp == 1),
                            perf_mode=DR,
                        )
                if VT_DRAIN == "act" or (VT_DRAIN == "alt" and g % 2 == 0):
                    nc.scalar.activation(
                        vt_sb[:, 4 * g : 4 * g + 4, :, 0:CH],
                        vtt.rearrange("p a (h c) -> p a h c", h=HL),
                        AF.Identity,
                    )
                else:
                    nc.vector.tensor_copy(
                        vt_sb[:, 4 * g : 4 * g + 4, :, 0:CH],
                        vtt.rearrange("p a (h c) -> p a h c", h=HL),
                    )

            # qk for the FIRST chunks only (k_a/q_a tc2=0); the remaining
            # 6 qk groups and the 4 vt groups are injected INTO the
            # attention stream (QK_STREAM=1) so the first score matmul
            # fires as soon as the first x half + wqk land, instead of
            # after all 32 qk matmuls.
            QK_STREAM = int(os.environ.get("QK_STREAM", "0"))
            qk_group(2, 0)                 # k_a s 0..1024
            qk_group(0, 0)                 # q_a t 0..1024
            if QK_STREAM:
                stream_extras = {
                    0: lambda: qk_group(2, 1),   # k_a s 1024.. (need g4)
                    1: lambda: vt_group4(0),     # sc 0-3      (need g5)
                    2: lambda: qk_group(3, 0),   # k_b         (need g16)
                    3: lambda: qk_group(1, 0),   # q_b         (need g16)
                    4: lambda: vt_group4(1),     # sc 4-7      (need g7)
                    5: lambda: qk_group(0, 1),   # q_a th1     (need g32)
                    6: lambda: vt_group4(2),     # sc 8-11     (need g9)
                    7: lambda: vt_group4(3),     # sc 12-15    (need g11)
                    8: lambda: qk_group(3, 1),   # k_b s 1024..(need g20)
                    9: lambda: qk_group(1, 1),   # q_b th1     (need g48)
                }
            else:
                qk_group(2, 1)
                qk_group(0, 1)
                for tc2 in range(2):
                    qk_group(3, tc2)
                    qk_group(1, tc2)
                stream_extras = {}
                if VT_POS == "pre":
                    for g in range(4):
                        vt_group4(g)

            # ---- attention ----
            a_sb = apool.tile([P, 2, T], FP8)

            # attention as a software-pipelined chunk stream: av matmuls
            # lag the scores/exp stream by AV_LAG chunk-pairs so PE never
            # waits on the previous unit's last exp at unit boundaries.
            AV_LAG = int(os.environ.get("AV_LAG", "6"))
            units = [(hd, th) for th in range(2) for hd in range(HL)]
            state = {}   # u -> dict(avs, w_ts)
            exp_ctr = [0]

            def unit_geom(u):
                hd, th = units[u]
                b0 = 64 * (hd % 2)
                q_mc = 0 if hd < 2 else 1
                k_mc = 2 if hd < 2 else 3
                return hd, th, b0, q_mc, k_mc

            def emit_chunk(u, scp):
                hd, th, b0, q_mc, k_mc = unit_geom(u)
                toff = th * TH
                if scp == 0:
                    state[u] = dict(
                        avs=ps_sh.tile([P, 2, 512], F32, tag="sh",
                                       name=f"av{hd}{th}"),
                        w_ts={})
                w_t = wpool.tile([P, 2, TH], FP8, name="wt")
                state[u]["w_ts"][scp] = w_t
                split = u >= len(units) - SPLIT_LAST
                for j in range(2):
                    sc = scp * 2 + j
                    sps = ps_sps.tile([P, TH], F32, tag="sps", name="sps")
                    for tq in range(2):
                        nc.tensor.matmul(
                            sps[:, tq * 512 : (tq + 1) * 512],
                            lhsT=qk_sb[b0 : b0 + CH, :, k_mc,
                                       sc * P : (sc + 1) * P],
                            rhs=qk_sb[b0 : b0 + CH, :, q_mc,
                                      toff + tq * 512 : toff + (tq + 1) * 512],
                            start=True, stop=True,
                            perf_mode=DR,
                        )
                    if split:
                        # tail units: halve each chunk across BOTH engines so
                        # the slot frees sooner and av-tq halves unblock early
                        nc.scalar.activation(
                            w_t[:, j, 0:512], sps[:, 0:512], AF.Exp,
                            scale=float(GAMMA))
                        nc.vector._custom_dve(
                            EXP4, out=w_t[:, j, 512:TH], in0=sps[:, 512:TH],
                            s0=float(QC0 * GAMMA), s1=float(QC1),
                            imm2=float(QC2))
                    elif (LAST_PAIR_SPLIT and u == len(units) - 1
                          and scp == 7):
                        # force the final chunk-pair onto BOTH engines so
                        # the last av fires as early as possible
                        if j == 0:
                            nc.scalar.activation(
                                w_t[:, j, :], sps, AF.Exp, scale=float(GAMMA))
                        else:
                            nc.vector._custom_dve(
                                EXP4, out=w_t[:, j, :], in0=sps,
                                s0=float(QC0 * GAMMA), s1=float(QC1),
                                imm2=float(QC2))
                    elif pat[exp_ctr[0]]:
                        nc.scalar.activation(
                            w_t[:, j, :], sps, AF.Exp, scale=float(GAMMA))
                    else:
                        nc.vector._custom_dve(
                            EXP4, out=w_t[:, j, :], in0=sps,
                            s0=float(QC0 * GAMMA), s1=float(QC1),
                            imm2=float(QC2))
                    exp_ctr[0] += 1

            def emit_av(u, scp):
                hd, th, b0, q_mc, k_mc = unit_geom(u)
                avs = state[u]["avs"]
                w_t = state[u]["w_ts"].pop(scp)
                for tq in range(2):
                    nc.tensor.matmul(
                        avs[:, tq, :],
                        lhsT=vt_sb[:, 2 * scp : 2 * scp + 2, hd, :],
                        rhs=w_t[:, :, tq * 512 : (tq + 1) * 512],
                        start=(scp == 0), stop=(scp == 7),
                        perf_mode=DR,
                    )

            def emit_normalize(u, between=None):
                hd, th, b0, q_mc, k_mc = unit_geom(u)
                toff = th * TH
                avs = state[u]["avs"]
                if between is None:
                    # fused across both tq halves: reciprocal of the 64
                    # replicated rowsum rows IS the broadcast recip; then
                    # one multiply [64,2,512]
                    rep = repp.tile([CH, 2, 512], F32, name="rep")
                    nc.vector.reciprocal(rep, avs[CH : 2 * CH, :, :])
                    nc.vector.tensor_tensor(
                        a_sb[CH * (hd % 2) : CH * (hd % 2) + CH, hd // 2,
                             toff : toff + TH],
                        avs[0:CH, :, :], rep, ALU.mult,
                    )
                else:
                    # last unit: split per tq so proj_tc(2) can interleave
                    for tq in range(2):
                        rep = repp.tile([CH, 512], F32, name="rep")
                        nc.vector.reciprocal(rep, avs[CH : 2 * CH, tq, :])
                        nc.vector.tensor_tensor(
                            a_sb[CH * (hd % 2) : CH * (hd % 2) + CH, hd // 2,
                                 toff + tq * 512 : toff + (tq + 1) * 512],
                            avs[0:CH, tq, :], rep, ALU.mult,
                        )
                        if tq == 0:
                            between()
                del state[u]

            def run_attention(extra=()):
                stream = [(u, scp) for u in range(len(units))
                          for scp in range(8)]
                norm_q = []   # units whose avs are done, normalize deferred

                def pop_norm():
                    lu = norm_q.pop(0)
                    emit_normalize(lu)
                    if lu == 3:           # last th0 unit done
                        proj_tc(0)
                    elif lu == 5:
                        proj_tc(1)

                for g, (u, scp) in enumerate(stream):
                    emit_chunk(u, scp)
                    if QK_STREAM:
                        if g in stream_extras:
                            stream_extras[g]()
                    elif VT_POS == "stream" and g < 4:
                        vt_group4(g)
                    elif VT_POS == "spread" and g in VT_SLOTS:
                        vt_group4(VT_SLOTS.index(g))
                    lag = g - AV_LAG
                    if lag >= 0:
                        lu, lscp = stream[lag]
                        emit_av(lu, lscp)
                        if lscp == 7:
                            norm_q.append(lu)
                    if norm_q:
                        lu = norm_q[0]
                        close_g = (lu * 8 + 7) + AV_LAG  # g when avs closed
                        if g >= close_g + NORM_LAG:
                            pop_norm()
                for lu, lscp in stream[-AV_LAG:]:
                    emit_av(lu, lscp)
                    if lscp == 7:
                        norm_q.append(lu)
                while len(norm_q) > 1:
                    pop_norm()
                emit_normalize(norm_q.pop(0),
                               between=lambda: proj_tc(2, tail=True))

            def proj_tc(tc4, tail=False):
                # oc-PAIR tiles with one fused ap-1024 drain each.  Tail
                # blocks (tc4 2,3) borrow the sps ring -- the score stream
                # is over, so its slots are free; early blocks follow
                # PJ_POOL (default: sh ring, whose slot just freed).
                for op2 in range(2):
                    if tail or PJ_POOL == 0:
                        pjt = ps_sps.tile([P, 2, 512], F32, tag="sps",
                                          name=f"pjs{tc4}{op2}")
                    elif PJ_POOL == 1:
                        pjt = ps_pj.tile([P, 2, 512], F32, tag="pj",
                                         name=f"pjs{tc4}{op2}")
                    else:
                        pjt = ps_sh.tile([P, 2, 512], F32, tag="sh",
                                         name=f"pjs{tc4}{op2}")
                    for o2 in range(2):
                        oc = op2 * 2 + o2
                        nc.tensor.matmul(
                            pjt[:, o2, :],
                            lhsT=wp[:, :, oc, :],
                            rhs=a_sb[:, :, tc4 * 512 : (tc4 + 1) * 512],
                            start=True, stop=True,
                            perf_mode=DR,
                        )
                    ot = outp.tile([P, 2, 512], BF16, name="otp")
                    if tc4 == 3 and tail and PJT_SPLIT_LAST:
                        # final block: halve each drain across both engines
                        # and ship each half as its own DMA so the last
                        # transfer starts as early as possible
                        for o2, eng in ((0, "act"), (1, "dve")):
                            if eng == "act":
                                nc.scalar.activation(
                                    ot[:, o2, :], pjt[:, o2, :], AF.Identity,
                                    scale=float(1.0 / (SV * SW)))
                            else:
                                nc.vector.tensor_scalar(
                                    ot[:, o2, :], pjt[:, o2, :],
                                    float(1.0 / (SV * SW)), None, ALU.mult)
                            nc.sync.dma_start(
                                out_d[:, 2 * op2 + o2,
                                      tc4 * 512 : (tc4 + 1) * 512],
                                ot[:, o2, :])
                        continue
                    if tc4 == 3 and tail and PJT3_DVE:
                        # final block: DVE frees first (after the norm
                        # chain); ACT is still on proj2's drains
                        on_act = op2 == 1
                    else:
                        on_act = (tail and PJT_TAIL_ACT) \
                            or PJ_DRAIN == "act" \
                            or (PJ_DRAIN == "alt" and op2 == 0)
                    if on_act:
                        nc.scalar.activation(
                            ot, pjt, AF.Identity,
                            scale=float(1.0 / (SV * SW)))
                    else:
                        nc.vector.tensor_scalar(
                            ot, pjt, float(1.0 / (SV * SW)),
                            None, ALU.mult)
                    nc.sync.dma_start(
                        out_d[:, 2 * op2 : 2 * op2 + 2,
                              tc4 * 512 : (tc4 + 1) * 512], ot)

            run_attention()
            proj_tc(3, tail=True)
    nc.compile()
    return nc


_NC = None
_LAST_RESULTS = None


def _get_nc():
    global _NC
    if _NC is None:
        _NC = _build_nc()
    return _NC


def _fp8(a):
    return np.ascontiguousarray(a.astype(np.float32).astype(E4))


def kernel(x, mask, gn_gamma, gn_beta, qkv_w, qkv_b, proj_w, proj_b,
           _trace=False):
    del mask  # all-True per problem spec
    x = np.asarray(x, np.float32)
    gn_gamma = np.asarray(gn_gamma, np.float32)
    gn_beta = np.asarray(gn_beta, np.float32)
    qkv_w = np.asarray(qkv_w, np.float32)
    qkv_b = np.asarray(qkv_b, np.float32)
    proj_w = np.asarray(proj_w, np.float32)
    proj_b = np.asarray(proj_b, np.float32)

    # exact GroupNorm stats per batch (host, f32)
    xg = x.reshape(B, G, C // G, T)
    mu = xg.mean(axis=(2, 3))                      # [B, G]
    var = xg.var(axis=(2, 3))                      # [B, G]
    s_bg = 1.0 / np.sqrt(var + EPS)                # [B, G]
    s_bc = np.repeat(s_bg, C // G, axis=1) * gn_gamma[None, :]      # [B, C]
    off_bc = gn_beta[None, :] - np.repeat(mu * s_bg, C // G, axis=1) \
        * gn_gamma[None, :]                        # [B, C]

    in_maps = []
    v_bias_term = {}
    for core in range(N_CORES):
        b, hh = core // 2, core % 2
        heads = [hh * HL + i for i in range(HL)]
        # column order for q/k: [head][ch]; mc blocks = head pairs
        q_rows = np.concatenate(
            [np.arange(h * 192, h * 192 + 64) for h in heads])
        k_rows = q_rows + 64
        v_rows = np.concatenate([np.arange(h * 192 + 128, h * 192 + 192)
                                 for h in heads])

        s = s_bc[b]                                # [C]
        off = off_bc[b]                            # [C]

        wq = qkv_w[q_rows] * s[None, :]            # [256, 512]
        wk = qkv_w[k_rows] * s[None, :]
        wv_ = qkv_w[v_rows] * s[None, :]
        # wqk dram layout [p(c%128), kc(c//128), mc, m(128)]
        wqk_m = np.concatenate([wq, wk], 0)        # [512(m), 512(c)]
        wqk_t = (wqk_m.T.reshape(4, P, 4, P)
                 .transpose(1, 0, 2, 3))           # [p, kc, mc, m]
        wqk_t = wqk_t * SW
        wv_t = wv_.T.reshape(4, P, 2 * P).transpose(1, 0, 2) * SW
        # proj columns for this half, reordered to head-band x ch
        wp_cols = proj_w[:, [hh * 256 + i for i in range(256)]]  # [512, 256]
        # a_sb rows: [hd%2 band (64), hd//2 ktile]: channel (hd, ch) sits at
        # row 64*(hd%2)+ch of ktile hd//2 -> input index hd*64+ch
        perm = np.array([(kt * 2 + band) * 64 + ch
                         for kt in range(2) for band in range(2)
                         for ch in range(64)])
        # rows of wp lhsT tile [p, kt, oc, m]: p = 64*band+ch
        wp_in = wp_cols[:, perm]                   # [512 out, 256 perm-in]
        wp_t = (wp_in.T.reshape(2, P, 4, P)
                .transpose(1, 0, 2, 3)) * SW       # [p, kt, oc, m]

        cq = (qkv_w[q_rows] @ off + qkv_b[q_rows]) * SCALE * SQ
        ck = (qkv_w[k_rows] @ off + qkv_b[k_rows]) * SCALE * SQ
        cqk = np.stack([cq[:P], cq[P:], ck[:P], ck[P:]], axis=1)  # [128, 4]

        x_t = x[b].reshape(4, P, T).transpose(1, 0, 2)

        in_maps.append(dict(
            x=_fp8(x_t),
            wqk=_fp8(wqk_t),
            wv=_fp8(wv_t),
            wp=_fp8(wp_t),
            cqk=np.ascontiguousarray(cqk, dtype=np.float32),
        ))
        # v bias + GN-offset contribution through v, exact on host:
        cv = qkv_w[v_rows] @ off + qkv_b[v_rows]   # [256]
        v_bias_term[core] = proj_w[:, hh * 256 : hh * 256 + 256] @ cv  # [512]

    nc = _get_nc()
    res = run_bass_kernel_spmd(nc, in_maps, core_ids=list(range(N_CORES)),
                               trace=_trace)
    global _LAST_RESULTS
    _LAST_RESULTS = res
    out = np.empty((B, C, T), np.float32)
    for b in range(B):
        r0 = res.results[2 * b]["out"].astype(np.float32)
        r1 = res.results[2 * b + 1]["out"].astype(np.float32)
        const = (v_bias_term[2 * b] + v_bias_term[2 * b + 1]
                 + proj_b)[:, None]
        out[b] = (x[b]
                  + r0.transpose(1, 0, 2).reshape(C, T)
                  + r1.transpose(1, 0, 2).reshape(C, T)
                  + const)
    return out
